# revision 5
# baseline (speedup 1.0000x reference)
"""ConvLSTM2D (filters=36, kernel 5x5, strides 4, valid) + Flatten on 8 trn2 cores.

Data-parallel over batch (B=8): core i handles batch element i end-to-end.

Per-core dataflow (all shapes per core):
  x: (16, 256, 256, 3) f32.  Wx: (75, 144), Wh: (900, 144), b: (144) f32.
  Input conv (stride 4, VALID) -> (63, 63, 144) per step, computed as one
  K=76 GEMM per 128-spatial block from a DMA-gathered im2col (bias folded
  into a ones-row).  Recurrent conv (SAME, stride 1) -> 10 accumulating
  GEMMs per block from a replicated channel-major padded h buffer (halo
  trick).  Gates are evaluated spatial-major ([128 positions, 144 cols]),
  LSTM state c kept in fp32, h written back channel-major via PE transpose.

Strip layout: o = y*67 + xo, y in [0,63), xo in [0,67); xo>=63 are halo
(garbage) columns that map exactly onto the zero-padding columns of the
padded h image, so they are memset to zero after each step.
"""

import os
import sys
import numpy as np

sys.path.insert(0, "/opt/trn_rl_repo")
sys.path.insert(0, "/opt/pypackages")

B = 8
T = int(os.environ.get("KERNEL_T", "16"))
H = W = 256
C = 3
F = 36
KK = 5
G4 = 4 * F            # 144
Ho = (H - KK) // 4 + 1  # 63
Ws = Ho + 4             # 67 strip cols (63 valid + 4 halo)
NY = Ho                 # 63 strip rows
S = NY * Ws             # 4221
SPAD = 4224             # 33 blocks of 128
NBLK = SPAD // 128      # 33
NG = 11                 # gate groups of 3 blocks
GSZ = 384               # strip positions per gate group
WC = W * C              # 768
RFREE = 4608            # R buffer free size (needs >= 4224 + 272 + margin)
ROFF = 136              # h strip position s lands at R0 column s + 136

# recurrent tap groups: (dy, dx0, K) ; lhsT = R[0:K, o + dy*67 + dx0 : +128]
REC_GROUPS = []
for dy in range(KK):
    REC_GROUPS.append((dy, 0, 108))
    REC_GROUPS.append((dy, 3, 72))

# gate column permutation: psum cols = [i(0:36) f(36:72) o(72:108) c~(108:144)]
# original keras order in the 144 axis: [i f c o]
# dest col range -> src col range in (Wx, Wh, b)
COL_PERM = [((0, 72), (0, 72)), ((72, 108), (108, 144)), ((108, 144), (72, 108))]


def garbage_runs_in(m0, m1):
    """R0 column runs (start, 4) inside [m0, m1) holding halo strip cols."""
    runs = []
    y = max(0, (m0 - ROFF - 63 + 66) // 67)
    while True:
        a = ROFF + y * 67 + 63
        if a >= m1:
            break
        if a >= m0:
            assert a + 4 <= m1, "garbage run straddles group boundary"
            runs.append(a)
        y += 1
    return runs


def build(nc_mod, bacc_mod, tile_mod, mybir, bass_mod, make_identity):
    nc = bacc_mod.Bacc("TRN2", target_bir_lowering=False, debug=False,
                       enable_asserts=True, num_devices=B)
    dt = mybir.dt
    Alu = mybir.AluOpType
    Act = mybir.ActivationFunctionType

    x_d = nc.dram_tensor("x", [T, H, W, C], dt.float32, kind="ExternalInput")
    wx_d = nc.dram_tensor("wx", [75, G4], dt.float32, kind="ExternalInput")
    wh_d = nc.dram_tensor("wh", [900, G4], dt.float32, kind="ExternalInput")
    b_d = nc.dram_tensor("b", [G4], dt.float32, kind="ExternalInput")
    out_d = nc.dram_tensor("out", [Ho * Ho * F], dt.float32, kind="ExternalOutput")
    stage_d = nc.dram_tensor("stage", [2, SPAD, 128], dt.bfloat16)
    hstage_d = nc.dram_tensor("hstage", [SPAD, F], dt.float32)

    with tile_mod.TileContext(nc) as tc:
        import contextlib
        with contextlib.ExitStack() as ctx:
            state = ctx.enter_context(tc.tile_pool(name="state", bufs=1))
            zpool = ctx.enter_context(tc.tile_pool(name="z2", bufs=4))
            ppool = ctx.enter_context(tc.tile_pool(name="prod", bufs=4))
            hpool = ctx.enter_context(tc.tile_pool(name="hsp", bufs=4))
            hfpool = ctx.enter_context(tc.tile_pool(name="hspf", bufs=4))
            gpsum = ctx.enter_context(tc.tile_pool(name="gps", bufs=3, space="PSUM"))
            tpsum = ctx.enter_context(tc.tile_pool(name="tps", bufs=2, space="PSUM"))

            # ---------------- persistent state ----------------
            X75 = [state.tile([128, SPAD], dt.bfloat16, name=f"x75_{i}") for i in range(2)]
            R = [state.tile([108, RFREE], dt.bfloat16, name=f"R{i}") for i in range(2)]
            tcst = state.tile([128, NBLK, 72], dt.float32, tag="tc")
            ident = state.tile([128, 128], dt.bfloat16, tag="ident")
            ones = state.tile([1, SPAD], dt.bfloat16, tag="ones")
            zrow = state.tile([1, 384], dt.bfloat16, tag="zrow")

            nc.vector.memset(R[0][:], 0.0)
            nc.vector.memset(R[1][:], 0.0)
            nc.vector.memset(tcst[:], 0.0)
            nc.vector.memset(ones[:], 1.0)
            nc.vector.memset(zrow[:], 0.0)
            make_identity(nc, ident[:])
            # zero the never-gathered tail rows (strip 4221..4223) of both
            # stage buffers: 3 rows x 128 cols = 384 bf16 elements each
            for i in range(2):
                nc.sync.dma_start(
                    out=bass_mod.AP(tensor=stage_d, offset=(i * SPAD + S) * 128,
                                    ap=[[1, (SPAD - S) * 128]]),
                    in_=zrow[0:1, 0:(SPAD - S) * 128])

            # ---------------- weights ----------------
            wxs = state.tile([76, G4], dt.float32, tag="wxs")
            wxg = state.tile([76, G4], dt.bfloat16, tag="wxg")
            for (d0, d1), (s0, s1) in COL_PERM:
                nc.gpsimd.dma_start(out=wxs[0:75, d0:d1], in_=wx_d[:, s0:s1])
            brow = state.tile([1, G4], dt.float32, tag="brow")
            for (d0, d1), (s0, s1) in COL_PERM:
                nc.gpsimd.dma_start(out=brow[0:1, d0:d1],
                                    in_=b_d[s0:s1].unsqueeze(0))
            # brow: sig cols -> 0.2*b + 0.5 ; c~ cols unchanged
            nc.vector.tensor_scalar(out=brow[:, 0:108], in0=brow[:, 0:108],
                                    scalar1=0.2, scalar2=0.5,
                                    op0=Alu.mult, op1=Alu.add)
            browb = state.tile([1, G4], dt.bfloat16, tag="browb")
            nc.vector.tensor_copy(browb[:], brow[:])
            nc.vector.tensor_scalar_mul(wxg[0:75, 0:108], wxs[0:75, 0:108], 0.2)
            nc.vector.tensor_copy(wxg[0:75, 108:144], wxs[0:75, 108:144])
            nc.sync.dma_start(out=wxg[75:76, :], in_=browb[:])

            whg = []
            whs = state.tile([108, G4], dt.float32, tag="whs")
            for gi, (dy, dx0, Kg) in enumerate(REC_GROUPS):
                r0 = dy * 180 + dx0 * 36
                wt = state.tile([Kg, G4], dt.bfloat16, name=f"whg{gi}")
                for (d0, d1), (s0, s1) in COL_PERM:
                    nc.gpsimd.dma_start(out=whs[0:Kg, d0:d1],
                                        in_=wh_d[r0:r0 + Kg, s0:s1])
                nc.vector.tensor_scalar_mul(wt[0:Kg, 0:108], whs[0:Kg, 0:108], 0.2)
                nc.vector.tensor_copy(wt[0:Kg, 108:144], whs[0:Kg, 108:144])
                whg.append(wt)

            # ---------------- im2col gather helpers ----------------
            def gather(t):
                sb = t % 2
                for dy in range(KK):
                    src = bass_mod.AP(
                        tensor=x_d,
                        offset=t * H * WC + dy * WC,
                        ap=[[4 * WC, NY], [4 * C, Ws], [1, 15]])
                    dst = bass_mod.AP(
                        tensor=stage_d,
                        offset=sb * SPAD * 128 + 15 * dy,
                        ap=[[128 * Ws, NY], [128, Ws], [1, 15]])
                    nc.gpsimd.dma_start(out=dst, in_=src)

            def transpose_x(t):
                sb = t % 2
                nc.sync.dma_start(out=X75[sb][:], in_=stage_d[sb], transpose=True)
                nc.sync.dma_start(out=X75[sb][75:76, :], in_=ones[:])

            gather(0)
            transpose_x(0)

            # ---------------- time steps ----------------
            for t in range(T):
                Xc = X75[t % 2]
                Rc = R[t % 2]
                Rn = R[(t + 1) % 2]
                last = (t == T - 1)
                if not last:
                    gather(t + 1)
                for g in range(NG):
                    ps = gpsum.tile([128, 3, G4], dt.float32, tag="gps")
                    for j in range(3):
                        blk = g * 3 + j
                        o0 = blk * 128
                        nc.tensor.matmul(ps[:, j, :], Xc[0:76, o0:o0 + 128], wxg[:],
                                         start=True, stop=(t == 0))
                        if t > 0:
                            for gi, (dy, dx0, Kg) in enumerate(REC_GROUPS):
                                off = o0 + dy * Ws + dx0
                                nc.tensor.matmul(
                                    ps[:, j, :], Rc[0:Kg, off:off + 128],
                                    whg[gi][:],
                                    start=False, stop=(gi == len(REC_GROUPS) - 1))
                    # ---- gates ----
                    z2 = zpool.tile([128, 3, 108], dt.float32, tag="z2")
                    nc.vector.tensor_scalar(out=z2[:], in0=ps[:, :, 0:108],
                                            scalar1=0.0, scalar2=1.0,
                                            op0=Alu.max, op1=Alu.min)
                    tslot = tcst[:, 3 * g:3 * g + 3, 0:36]
                    cslot = tcst[:, 3 * g:3 * g + 3, 36:72]
                    nc.scalar.activation(out=tslot, in_=ps[:, :, 108:144],
                                         func=Act.Tanh)
                    prod = ppool.tile([128, 3, 72], dt.float32, tag="prod")
                    nc.vector.tensor_tensor(out=prod[:], in0=z2[:, :, 0:72],
                                            in1=tcst[:, 3 * g:3 * g + 3, 0:72],
                                            op=Alu.mult)
                    nc.vector.tensor_tensor(out=cslot, in0=prod[:, :, 0:36],
                                            in1=prod[:, :, 36:72], op=Alu.add)
                    nc.scalar.activation(out=tslot, in_=cslot, func=Act.Tanh)
                    if last:
                        hf = hfpool.tile([128, 3, F], dt.float32, tag="hf")
                        nc.vector.tensor_tensor(out=hf[:], in0=z2[:, :, 72:108],
                                                in1=tslot, op=Alu.mult)
                        # hf flat order is (p, j, c); strip row = j*128 + p
                        nc.gpsimd.dma_start(
                            out=bass_mod.AP(tensor=hstage_d,
                                            offset=g * GSZ * F,
                                            ap=[[F, 128], [128 * F, 3], [1, F]]),
                            in_=hf[:])
                        continue
                    hsp = hpool.tile([128, 3, F], dt.bfloat16, tag="hsp")
                    nc.vector.tensor_tensor(out=hsp[:], in0=z2[:, :, 72:108],
                                            in1=tslot, op=Alu.mult)
                    # ---- transpose h back to channel-major into Rn ----
                    pt = tpsum.tile([F, GSZ], dt.bfloat16, tag="pt")
                    for j in range(3):
                        nc.tensor.transpose(pt[:, j * 128:(j + 1) * 128],
                                            hsp[:, j, :], ident[:])
                    m0 = ROFF + g * GSZ
                    m1 = min(m0 + GSZ, ROFF + S)
                    nc.scalar.copy(Rn[0:F, m0:m1], pt[:, 0:m1 - m0])
                    for a in garbage_runs_in(m0, m1):
                        nc.gpsimd.memset(
                            bass_mod.AP(tensor=Rn.tensor,
                                        offset=Rn.offset + a,
                                        ap=[[RFREE, F], [1, 4]]), 0.0)
                    for grp in (1, 2):
                        nc.sync.dma_start(
                            out=Rn[F * grp:F * (grp + 1), m0 - grp:m1 - grp],
                            in_=Rn[0:F, m0:m1])
                if not last:
                    transpose_x(t + 1)

            # ---------------- final output ----------------
            src = bass_mod.AP(tensor=hstage_d, offset=0,
                              ap=[[Ws * F, Ho], [1, Ho * F]])
            dst = bass_mod.AP(tensor=out_d, offset=0,
                              ap=[[Ho * F, Ho], [1, Ho * F]])
            nc.sync.dma_start(out=dst, in_=src)

    nc.compile()
    return nc


_CACHE = {}


def _get_nc():
    if "nc" not in _CACHE:
        import concourse.bass as bass_mod
        import concourse.bacc as bacc_mod
        import concourse.tile as tile_mod
        from concourse import mybir
        from concourse.masks import make_identity
        _CACHE["nc"] = build(bass_mod, bacc_mod, tile_mod, mybir, bass_mod,
                             make_identity)
    return _CACHE["nc"]


def run_on_hw(states_in, Wx, Wh, b, trace=False):
    from concourse.bass_utils import run_bass_kernel_spmd
    nc = _get_nc()
    wx = np.ascontiguousarray(Wx.reshape(75, G4), dtype=np.float32)
    wh = np.ascontiguousarray(Wh.reshape(900, G4), dtype=np.float32)
    bb = np.ascontiguousarray(b, dtype=np.float32)
    in_maps = [{
        "x": np.ascontiguousarray(states_in[i, :T], dtype=np.float32),
        "wx": wx, "wh": wh, "b": bb,
    } for i in range(B)]
    res = run_bass_kernel_spmd(nc, in_maps, core_ids=list(range(B)),
                               trace=trace)
    out = np.stack([res.results[i]["out"] for i in range(B)])
    return out, res


def kernel(states_in, Wx, Wh, b):
    out, _ = run_on_hw(states_in, Wx, Wh, b)
    return out.astype(np.float32)


# ---------------------------------------------------------------------------
# numpy simulation of the exact same dataflow (for offline logic validation)
# ---------------------------------------------------------------------------
def numpy_sim(states_in, Wx, Wh, b, bf16=True):
    import ml_dtypes

    def q(a):
        return a.astype(ml_dtypes.bfloat16).astype(np.float32) if bf16 else a

    out = np.zeros((B, Ho * Ho * F), np.float32)
    wx = Wx.reshape(75, G4).astype(np.float32)
    wh = Wh.reshape(900, G4).astype(np.float32)

    # permuted + scaled weights
    def permcols(a):
        r = np.zeros_like(a)
        for (d0, d1), (s0, s1) in COL_PERM:
            r[..., d0:d1] = a[..., s0:s1]
        return r

    wxp = permcols(wx)
    whp = permcols(wh)
    bp = permcols(b.astype(np.float32))
    brow = np.concatenate([0.2 * bp[0:108] + 0.5, bp[108:144]])
    wxg = np.vstack([wxp, brow[None, :]])
    wxg[0:75, 0:108] *= 0.2
    wxg = q(wxg)
    whgs = []
    for dy, dx0, Kg in REC_GROUPS:
        r0 = dy * 180 + dx0 * 36
        wt = whp[r0:r0 + Kg].copy()
        wt[:, 0:108] *= 0.2
        whgs.append(q(wt))

    ys, xos = np.divmod(np.arange(S), Ws)
    i0 = (4 * ys[:, None] + np.arange(KK)[None, :]) * WC + 4 * xos[:, None] * C
    gidx = i0[:, :, None] + np.arange(15)[None, None, :]  # [S, 5, 15]

    for bi in range(B):
        xflat = states_in[bi, :T].astype(np.float32).reshape(T, -1)
        R0 = [np.zeros((108, RFREE), np.float32) for _ in range(2)]
        cst = np.zeros((SPAD, F), np.float32)
        hfinal = np.zeros((SPAD, F), np.float32)

        def build_x75(t):
            X = np.zeros((76, SPAD), np.float32)
            X[75] = 1.0
            vals = q(xflat[t][gidx])            # [S, 5, 15]
            X[0:75, 0:S] = vals.reshape(S, 75).T
            return X

        for t in range(T):
            Xc = build_x75(t)
            Rc = R0[t % 2]
            Rn = R0[(t + 1) % 2]
            last = (t == T - 1)
            h_all = np.zeros((SPAD, F), np.float32)
            for blk in range(NBLK):
                o0 = blk * 128
                ps = Xc[:, o0:o0 + 128].T @ wxg
                if t > 0:
                    for gi, (dy, dx0, Kg) in enumerate(REC_GROUPS):
                        off = o0 + dy * Ws + dx0
                        ps = ps + Rc[0:Kg, off:off + 128].T @ whgs[gi]
                sig = np.clip(ps[:, 0:108], 0.0, 1.0)
                tta = np.tanh(ps[:, 108:144])
                i_g, f_g, o_g = sig[:, 0:36], sig[:, 36:72], sig[:, 72:108]
                c_old = cst[o0:o0 + 128]
                c_new = f_g * c_old + i_g * tta
                cst[o0:o0 + 128] = c_new
                h_all[o0:o0 + 128] = o_g * np.tanh(c_new)
            if last:
                hfinal = h_all
            else:
                Rn[:] = 0.0
                Rn[0:F, ROFF:ROFF + S] = q(h_all[:S]).T
                mask = np.ones(S, bool)
                mask[xos >= 63] = False
                Rn[0:F, ROFF:ROFF + S][:, ~mask] = 0.0
                Rn[F:2 * F, :-1] = Rn[0:F, 1:]
                Rn[2 * F:3 * F, :-2] = Rn[0:F, 2:]

        hf = hfinal[:S].reshape(NY, Ws, F)[:, 0:Ho, :]
        out[bi] = hf.reshape(-1)
    return out


# revision 6
# speedup vs baseline: 69.1920x; 69.1920x over previous
"""ConvLSTM2D (filters=36, kernel 5x5, strides 4, valid) + Flatten on 8 trn2 cores.

Data-parallel over batch (B=8): core i handles batch element i end-to-end.

Per-core dataflow (all shapes per core):
  x: (16, 256, 256, 3) f32.  Wx: (75, 144), Wh: (900, 144), b: (144) f32.
  Input conv (stride 4, VALID) -> (63, 63, 144) per step, computed as one
  K=76 GEMM per 128-spatial block from a DMA-gathered im2col (bias folded
  into a ones-row).  Recurrent conv (SAME, stride 1) -> 10 accumulating
  GEMMs per block from a replicated channel-major padded h buffer (halo
  trick).  Gates are evaluated spatial-major ([128 positions, 144 cols]),
  LSTM state c kept in fp32, h written back channel-major via PE transpose.

Strip layout: o = y*67 + xo, y in [0,63), xo in [0,67); xo>=63 are halo
(garbage) columns that map exactly onto the zero-padding columns of the
padded h image, so they are memset to zero after each step.
"""

import os
import sys
import numpy as np

sys.path.insert(0, "/opt/trn_rl_repo")
sys.path.insert(0, "/opt/pypackages")

# The kernel executes on the 8 axon-tunneled NeuronCores via PJRT. If the
# calling process pinned jax to cpu (common for running the jax reference)
# and jax has not been imported yet, drop the pin so the axon platform is
# discoverable when the bass runtime initializes jax.
if "jax" not in sys.modules and os.environ.get("JAX_PLATFORMS") == "cpu":
    del os.environ["JAX_PLATFORMS"]

B = 8
T = int(os.environ.get("KERNEL_T", "16"))
H = W = 256
C = 3
F = 36
KK = 5
G4 = 4 * F            # 144
Ho = (H - KK) // 4 + 1  # 63
Ws = Ho + 4             # 67 strip cols (63 valid + 4 halo)
NY = Ho                 # 63 strip rows
S = NY * Ws             # 4221
SPAD = 4224             # 33 blocks of 128
NBLK = SPAD // 128      # 33
NG = 11                 # gate groups of 3 blocks
GSZ = 384               # strip positions per gate group
WC = W * C              # 768
RFREE = 4608            # R buffer free size (needs >= 4224 + 272 + margin)
ROFF = 136              # h strip position s lands at R0 column s + 136

# recurrent tap groups: (dy, dx0, K) ; lhsT = R[0:K, o + dy*67 + dx0 : +128]
REC_GROUPS = []
for dy in range(KK):
    REC_GROUPS.append((dy, 0, 108))
    REC_GROUPS.append((dy, 3, 72))

# gate column permutation: psum cols = [i(0:36) f(36:72) o(72:108) c~(108:144)]
# original keras order in the 144 axis: [i f c o]
# dest col range -> src col range in (Wx, Wh, b)
COL_PERM = [((0, 72), (0, 72)), ((72, 108), (108, 144)), ((108, 144), (72, 108))]


def garbage_runs_in(m0, m1):
    """R0 column runs (start, 4) inside [m0, m1) holding halo strip cols."""
    runs = []
    y = max(0, (m0 - ROFF - 63 + 66) // 67)
    while True:
        a = ROFF + y * 67 + 63
        if a >= m1:
            break
        if a >= m0:
            assert a + 4 <= m1, "garbage run straddles group boundary"
            runs.append(a)
        y += 1
    return runs


def build(nc_mod, bacc_mod, tile_mod, mybir, bass_mod, make_identity):
    nc = bacc_mod.Bacc("TRN2", target_bir_lowering=False, debug=False,
                       enable_asserts=True, num_devices=B)
    dt = mybir.dt
    Alu = mybir.AluOpType
    Act = mybir.ActivationFunctionType

    x_d = nc.dram_tensor("x", [T, H, W, C], dt.float32, kind="ExternalInput")
    wx_d = nc.dram_tensor("wx", [75, G4], dt.float32, kind="ExternalInput")
    wh_d = nc.dram_tensor("wh", [900, G4], dt.float32, kind="ExternalInput")
    b_d = nc.dram_tensor("b", [G4], dt.float32, kind="ExternalInput")
    out_d = nc.dram_tensor("out", [Ho * Ho * F], dt.float32, kind="ExternalOutput")
    stage_d = nc.dram_tensor("stage", [2, SPAD, 128], dt.bfloat16)
    hstage_d = nc.dram_tensor("hstage", [SPAD, F], dt.float32)

    with tile_mod.TileContext(nc) as tc:
        import contextlib
        with contextlib.ExitStack() as ctx:
            state = ctx.enter_context(tc.tile_pool(name="state", bufs=1))
            zpool = ctx.enter_context(tc.tile_pool(name="z2", bufs=4))
            ppool = ctx.enter_context(tc.tile_pool(name="prod", bufs=4))
            hpool = ctx.enter_context(tc.tile_pool(name="hsp", bufs=4))
            hfpool = ctx.enter_context(tc.tile_pool(name="hspf", bufs=4))
            gpsum = ctx.enter_context(tc.tile_pool(name="gps", bufs=3, space="PSUM"))
            tpsum = ctx.enter_context(tc.tile_pool(name="tps", bufs=2, space="PSUM"))

            # ---------------- persistent state ----------------
            X75 = [state.tile([128, SPAD], dt.bfloat16, name=f"x75_{i}") for i in range(2)]
            R = [state.tile([108, RFREE], dt.bfloat16, name=f"R{i}") for i in range(2)]
            tcst = state.tile([128, NBLK, 72], dt.float32, tag="tc")
            ident = state.tile([128, 128], dt.bfloat16, tag="ident")
            ones = state.tile([1, SPAD], dt.bfloat16, tag="ones")
            zrow = state.tile([1, 384], dt.bfloat16, tag="zrow")

            nc.vector.memset(R[0][:], 0.0)
            nc.vector.memset(R[1][:], 0.0)
            nc.vector.memset(tcst[:], 0.0)
            nc.vector.memset(ones[:], 1.0)
            nc.vector.memset(zrow[:], 0.0)
            make_identity(nc, ident[:])
            # zero the never-gathered tail rows (strip 4221..4223) of both
            # stage buffers: 3 rows x 128 cols = 384 bf16 elements each
            for i in range(2):
                nc.sync.dma_start(
                    out=bass_mod.AP(tensor=stage_d, offset=(i * SPAD + S) * 128,
                                    ap=[[1, (SPAD - S) * 128]]),
                    in_=zrow[0:1, 0:(SPAD - S) * 128])

            # ---------------- weights ----------------
            wxs = state.tile([76, G4], dt.float32, tag="wxs")
            wxg = state.tile([76, G4], dt.bfloat16, tag="wxg")
            for (d0, d1), (s0, s1) in COL_PERM:
                nc.gpsimd.dma_start(out=wxs[0:75, d0:d1], in_=wx_d[:, s0:s1])
            brow = state.tile([1, G4], dt.float32, tag="brow")
            for (d0, d1), (s0, s1) in COL_PERM:
                nc.gpsimd.dma_start(out=brow[0:1, d0:d1],
                                    in_=b_d[s0:s1].unsqueeze(0))
            # brow: sig cols -> 0.2*b + 0.5 ; c~ cols unchanged
            nc.vector.tensor_scalar(out=brow[:, 0:108], in0=brow[:, 0:108],
                                    scalar1=0.2, scalar2=0.5,
                                    op0=Alu.mult, op1=Alu.add)
            browb = state.tile([1, G4], dt.bfloat16, tag="browb")
            nc.vector.tensor_copy(browb[:], brow[:])
            nc.vector.tensor_scalar_mul(wxg[0:75, 0:108], wxs[0:75, 0:108], 0.2)
            nc.vector.tensor_copy(wxg[0:75, 108:144], wxs[0:75, 108:144])
            nc.sync.dma_start(out=wxg[75:76, :], in_=browb[:])

            whg = []
            whs = state.tile([108, G4], dt.float32, tag="whs")
            for gi, (dy, dx0, Kg) in enumerate(REC_GROUPS):
                r0 = dy * 180 + dx0 * 36
                wt = state.tile([Kg, G4], dt.bfloat16, name=f"whg{gi}")
                for (d0, d1), (s0, s1) in COL_PERM:
                    nc.gpsimd.dma_start(out=whs[0:Kg, d0:d1],
                                        in_=wh_d[r0:r0 + Kg, s0:s1])
                nc.vector.tensor_scalar_mul(wt[0:Kg, 0:108], whs[0:Kg, 0:108], 0.2)
                nc.vector.tensor_copy(wt[0:Kg, 108:144], whs[0:Kg, 108:144])
                whg.append(wt)

            # ---------------- im2col gather helpers ----------------
            def gather(t):
                sb = t % 2
                for dy in range(KK):
                    src = bass_mod.AP(
                        tensor=x_d,
                        offset=t * H * WC + dy * WC,
                        ap=[[4 * WC, NY], [4 * C, Ws], [1, 15]])
                    dst = bass_mod.AP(
                        tensor=stage_d,
                        offset=sb * SPAD * 128 + 15 * dy,
                        ap=[[128 * Ws, NY], [128, Ws], [1, 15]])
                    nc.gpsimd.dma_start(out=dst, in_=src)

            def transpose_x(t):
                sb = t % 2
                nc.sync.dma_start(out=X75[sb][:], in_=stage_d[sb], transpose=True)
                nc.sync.dma_start(out=X75[sb][75:76, :], in_=ones[:])

            gather(0)
            transpose_x(0)

            # ---------------- time steps ----------------
            for t in range(T):
                Xc = X75[t % 2]
                Rc = R[t % 2]
                Rn = R[(t + 1) % 2]
                last = (t == T - 1)
                if not last:
                    gather(t + 1)
                for g in range(NG):
                    ps = gpsum.tile([128, 3, G4], dt.float32, tag="gps")
                    for j in range(3):
                        blk = g * 3 + j
                        o0 = blk * 128
                        nc.tensor.matmul(ps[:, j, :], Xc[0:76, o0:o0 + 128], wxg[:],
                                         start=True, stop=(t == 0))
                        if t > 0:
                            for gi, (dy, dx0, Kg) in enumerate(REC_GROUPS):
                                off = o0 + dy * Ws + dx0
                                nc.tensor.matmul(
                                    ps[:, j, :], Rc[0:Kg, off:off + 128],
                                    whg[gi][:],
                                    start=False, stop=(gi == len(REC_GROUPS) - 1))
                    # ---- gates ----
                    z2 = zpool.tile([128, 3, 108], dt.float32, tag="z2")
                    nc.vector.tensor_scalar(out=z2[:], in0=ps[:, :, 0:108],
                                            scalar1=0.0, scalar2=1.0,
                                            op0=Alu.max, op1=Alu.min)
                    tslot = tcst[:, 3 * g:3 * g + 3, 0:36]
                    cslot = tcst[:, 3 * g:3 * g + 3, 36:72]
                    nc.scalar.activation(out=tslot, in_=ps[:, :, 108:144],
                                         func=Act.Tanh)
                    prod = ppool.tile([128, 3, 72], dt.float32, tag="prod")
                    nc.vector.tensor_tensor(out=prod[:], in0=z2[:, :, 0:72],
                                            in1=tcst[:, 3 * g:3 * g + 3, 0:72],
                                            op=Alu.mult)
                    nc.vector.tensor_tensor(out=cslot, in0=prod[:, :, 0:36],
                                            in1=prod[:, :, 36:72], op=Alu.add)
                    nc.scalar.activation(out=tslot, in_=cslot, func=Act.Tanh)
                    if last:
                        hf = hfpool.tile([128, 3, F], dt.float32, tag="hf")
                        nc.vector.tensor_tensor(out=hf[:], in0=z2[:, :, 72:108],
                                                in1=tslot, op=Alu.mult)
                        # hf flat order is (p, j, c); strip row = j*128 + p
                        nc.gpsimd.dma_start(
                            out=bass_mod.AP(tensor=hstage_d,
                                            offset=g * GSZ * F,
                                            ap=[[F, 128], [128 * F, 3], [1, F]]),
                            in_=hf[:])
                        continue
                    hsp = hpool.tile([128, 3, F], dt.bfloat16, tag="hsp")
                    nc.vector.tensor_tensor(out=hsp[:], in0=z2[:, :, 72:108],
                                            in1=tslot, op=Alu.mult)
                    # ---- transpose h back to channel-major into Rn ----
                    pt = tpsum.tile([F, GSZ], dt.bfloat16, tag="pt")
                    for j in range(3):
                        nc.tensor.transpose(pt[:, j * 128:(j + 1) * 128],
                                            hsp[:, j, :], ident[:])
                    m0 = ROFF + g * GSZ
                    m1 = min(m0 + GSZ, ROFF + S)
                    nc.scalar.copy(Rn[0:F, m0:m1], pt[:, 0:m1 - m0])
                    for a in garbage_runs_in(m0, m1):
                        nc.gpsimd.memset(
                            bass_mod.AP(tensor=Rn.tensor,
                                        offset=Rn.offset + a,
                                        ap=[[RFREE, F], [1, 4]]), 0.0)
                    for grp in (1, 2):
                        nc.sync.dma_start(
                            out=Rn[F * grp:F * (grp + 1), m0 - grp:m1 - grp],
                            in_=Rn[0:F, m0:m1])
                if not last:
                    transpose_x(t + 1)

            # ---------------- final output ----------------
            src = bass_mod.AP(tensor=hstage_d, offset=0,
                              ap=[[Ws * F, Ho], [1, Ho * F]])
            dst = bass_mod.AP(tensor=out_d, offset=0,
                              ap=[[Ho * F, Ho], [1, Ho * F]])
            nc.sync.dma_start(out=dst, in_=src)

    nc.compile()
    return nc


_CACHE = {}


def _get_nc():
    if "nc" not in _CACHE:
        import concourse.bass as bass_mod
        import concourse.bacc as bacc_mod
        import concourse.tile as tile_mod
        from concourse import mybir
        from concourse.masks import make_identity
        _CACHE["nc"] = build(bass_mod, bacc_mod, tile_mod, mybir, bass_mod,
                             make_identity)
    return _CACHE["nc"]


def run_on_hw(states_in, Wx, Wh, b, trace=False):
    from concourse.bass_utils import run_bass_kernel_spmd
    nc = _get_nc()
    wx = np.ascontiguousarray(Wx.reshape(75, G4), dtype=np.float32)
    wh = np.ascontiguousarray(Wh.reshape(900, G4), dtype=np.float32)
    bb = np.ascontiguousarray(b, dtype=np.float32)
    in_maps = [{
        "x": np.ascontiguousarray(states_in[i, :T], dtype=np.float32),
        "wx": wx, "wh": wh, "b": bb,
    } for i in range(B)]
    res = run_bass_kernel_spmd(nc, in_maps, core_ids=list(range(B)),
                               trace=trace)
    out = np.stack([res.results[i]["out"] for i in range(B)])
    return out, res


def kernel(states_in, Wx, Wh, b):
    out, _ = run_on_hw(states_in, Wx, Wh, b)
    return out.astype(np.float32)


# ---------------------------------------------------------------------------
# numpy simulation of the exact same dataflow (for offline logic validation)
# ---------------------------------------------------------------------------
def numpy_sim(states_in, Wx, Wh, b, bf16=True):
    import ml_dtypes

    def q(a):
        return a.astype(ml_dtypes.bfloat16).astype(np.float32) if bf16 else a

    out = np.zeros((B, Ho * Ho * F), np.float32)
    wx = Wx.reshape(75, G4).astype(np.float32)
    wh = Wh.reshape(900, G4).astype(np.float32)

    # permuted + scaled weights
    def permcols(a):
        r = np.zeros_like(a)
        for (d0, d1), (s0, s1) in COL_PERM:
            r[..., d0:d1] = a[..., s0:s1]
        return r

    wxp = permcols(wx)
    whp = permcols(wh)
    bp = permcols(b.astype(np.float32))
    brow = np.concatenate([0.2 * bp[0:108] + 0.5, bp[108:144]])
    wxg = np.vstack([wxp, brow[None, :]])
    wxg[0:75, 0:108] *= 0.2
    wxg = q(wxg)
    whgs = []
    for dy, dx0, Kg in REC_GROUPS:
        r0 = dy * 180 + dx0 * 36
        wt = whp[r0:r0 + Kg].copy()
        wt[:, 0:108] *= 0.2
        whgs.append(q(wt))

    ys, xos = np.divmod(np.arange(S), Ws)
    i0 = (4 * ys[:, None] + np.arange(KK)[None, :]) * WC + 4 * xos[:, None] * C
    gidx = i0[:, :, None] + np.arange(15)[None, None, :]  # [S, 5, 15]

    for bi in range(B):
        xflat = states_in[bi, :T].astype(np.float32).reshape(T, -1)
        R0 = [np.zeros((108, RFREE), np.float32) for _ in range(2)]
        cst = np.zeros((SPAD, F), np.float32)
        hfinal = np.zeros((SPAD, F), np.float32)

        def build_x75(t):
            X = np.zeros((76, SPAD), np.float32)
            X[75] = 1.0
            vals = q(xflat[t][gidx])            # [S, 5, 15]
            X[0:75, 0:S] = vals.reshape(S, 75).T
            return X

        for t in range(T):
            Xc = build_x75(t)
            Rc = R0[t % 2]
            Rn = R0[(t + 1) % 2]
            last = (t == T - 1)
            h_all = np.zeros((SPAD, F), np.float32)
            for blk in range(NBLK):
                o0 = blk * 128
                ps = Xc[:, o0:o0 + 128].T @ wxg
                if t > 0:
                    for gi, (dy, dx0, Kg) in enumerate(REC_GROUPS):
                        off = o0 + dy * Ws + dx0
                        ps = ps + Rc[0:Kg, off:off + 128].T @ whgs[gi]
                sig = np.clip(ps[:, 0:108], 0.0, 1.0)
                tta = np.tanh(ps[:, 108:144])
                i_g, f_g, o_g = sig[:, 0:36], sig[:, 36:72], sig[:, 72:108]
                c_old = cst[o0:o0 + 128]
                c_new = f_g * c_old + i_g * tta
                cst[o0:o0 + 128] = c_new
                h_all[o0:o0 + 128] = o_g * np.tanh(c_new)
            if last:
                hfinal = h_all
            else:
                Rn[:] = 0.0
                Rn[0:F, ROFF:ROFF + S] = q(h_all[:S]).T
                mask = np.ones(S, bool)
                mask[xos >= 63] = False
                Rn[0:F, ROFF:ROFF + S][:, ~mask] = 0.0
                Rn[F:2 * F, :-1] = Rn[0:F, 1:]
                Rn[2 * F:3 * F, :-2] = Rn[0:F, 2:]

        hf = hfinal[:S].reshape(NY, Ws, F)[:, 0:Ho, :]
        out[bi] = hf.reshape(-1)
    return out


# revision 7
# speedup vs baseline: 70.2470x; 1.0152x over previous
"""ConvLSTM2D (filters=36, kernel 5x5, strides 4, valid) + Flatten on 8 trn2 cores.

Data-parallel over batch (B=8): core i handles batch element i end-to-end.

Per-core dataflow (all shapes per core):
  x: (16, 256, 256, 3) f32.  Wx: (75, 144), Wh: (900, 144), b: (144) f32.
  Input conv (stride 4, VALID) -> (63, 63, 144) per step, computed as one
  K=76 GEMM per 128-spatial block from a DMA-gathered im2col (bias folded
  into a ones-row).  Recurrent conv (SAME, stride 1) -> 10 accumulating
  GEMMs per block from a replicated channel-major padded h buffer (halo
  trick).  Gates are evaluated spatial-major ([128 positions, 144 cols]),
  LSTM state c kept in fp32, h written back channel-major via PE transpose.

Strip layout: o = y*67 + xo, y in [0,63), xo in [0,67); xo>=63 are halo
(garbage) columns that map exactly onto the zero-padding columns of the
padded h image, so they are memset to zero after each step.
"""

import os
import sys
import numpy as np

sys.path.insert(0, "/opt/trn_rl_repo")
sys.path.insert(0, "/opt/pypackages")

# The kernel executes on the 8 axon-tunneled NeuronCores via PJRT. If the
# calling process pinned jax to cpu (common for running the jax reference)
# and jax has not been imported yet, drop the pin so the axon platform is
# discoverable when the bass runtime initializes jax.
if "jax" not in sys.modules and os.environ.get("JAX_PLATFORMS") == "cpu":
    del os.environ["JAX_PLATFORMS"]

B = 8
T = int(os.environ.get("KERNEL_T", "16"))
H = W = 256
C = 3
F = 36
KK = 5
G4 = 4 * F            # 144
Ho = (H - KK) // 4 + 1  # 63
Ws = Ho + 4             # 67 strip cols (63 valid + 4 halo)
NY = Ho                 # 63 strip rows
S = NY * Ws             # 4221
SPAD = 4224             # 33 blocks of 128
NBLK = SPAD // 128      # 33
NG = 11                 # gate groups of 3 blocks
GSZ = 384               # strip positions per gate group
WC = W * C              # 768
RFREE = 4608            # R buffer free size (needs >= 4224 + 272 + margin)
ROFF = 136              # h strip position s lands at R0 column s + 136

# recurrent tap groups: (dy, dx0, K) ; lhsT = R[0:K, o + dy*67 + dx0 : +128]
REC_GROUPS = []
for dy in range(KK):
    REC_GROUPS.append((dy, 0, 108))
    REC_GROUPS.append((dy, 3, 72))

# gate column permutation: psum cols = [i(0:36) f(36:72) o(72:108) c~(108:144)]
# original keras order in the 144 axis: [i f c o]
# dest col range -> src col range in (Wx, Wh, b)
COL_PERM = [((0, 72), (0, 72)), ((72, 108), (108, 144)), ((108, 144), (72, 108))]


def garbage_runs_in(m0, m1):
    """R0 column runs (start, 4) inside [m0, m1) holding halo strip cols."""
    runs = []
    y = max(0, (m0 - ROFF - 63 + 66) // 67)
    while True:
        a = ROFF + y * 67 + 63
        if a >= m1:
            break
        if a >= m0:
            assert a + 4 <= m1, "garbage run straddles group boundary"
            runs.append(a)
        y += 1
    return runs


def build(nc_mod, bacc_mod, tile_mod, mybir, bass_mod, make_identity):
    nc = bacc_mod.Bacc("TRN2", target_bir_lowering=False, debug=False,
                       enable_asserts=True, num_devices=B)
    dt = mybir.dt
    Alu = mybir.AluOpType
    Act = mybir.ActivationFunctionType

    x_d = nc.dram_tensor("x", [T, H, W, C], dt.float32, kind="ExternalInput")
    wx_d = nc.dram_tensor("wx", [75, G4], dt.float32, kind="ExternalInput")
    wh_d = nc.dram_tensor("wh", [900, G4], dt.float32, kind="ExternalInput")
    b_d = nc.dram_tensor("b", [G4], dt.float32, kind="ExternalInput")
    out_d = nc.dram_tensor("out", [Ho * Ho * F], dt.float32, kind="ExternalOutput")
    stage_d = nc.dram_tensor("stage", [2, SPAD, 128], dt.bfloat16)
    hstage_d = nc.dram_tensor("hstage", [SPAD, F], dt.float32)

    with tile_mod.TileContext(nc) as tc:
        import contextlib
        with contextlib.ExitStack() as ctx:
            state = ctx.enter_context(tc.tile_pool(name="state", bufs=1))
            zpool = ctx.enter_context(tc.tile_pool(name="z2", bufs=6))
            ppool = ctx.enter_context(tc.tile_pool(name="prod", bufs=6))
            hpool = ctx.enter_context(tc.tile_pool(name="hsp", bufs=6))
            hfpool = ctx.enter_context(tc.tile_pool(name="hspf", bufs=4))
            gpsum = ctx.enter_context(tc.tile_pool(name="gps", bufs=4, space="PSUM"))
            tpsum = ctx.enter_context(tc.tile_pool(name="tps", bufs=3, space="PSUM"))

            # ---------------- persistent state ----------------
            X75 = [state.tile([128, SPAD], dt.bfloat16, name=f"x75_{i}") for i in range(2)]
            R = [state.tile([108, RFREE], dt.bfloat16, name=f"R{i}") for i in range(2)]
            tcst = state.tile([128, NBLK, 72], dt.float32, tag="tc")
            ident = state.tile([128, 128], dt.bfloat16, tag="ident")
            ones = state.tile([1, SPAD], dt.bfloat16, tag="ones")
            zrow = state.tile([1, 384], dt.bfloat16, tag="zrow")

            nc.vector.memset(R[0][:], 0.0)
            nc.vector.memset(R[1][:], 0.0)
            nc.vector.memset(tcst[:], 0.0)
            nc.vector.memset(ones[:], 1.0)
            nc.vector.memset(zrow[:], 0.0)
            make_identity(nc, ident[:])
            # zero the never-gathered tail rows (strip 4221..4223) of both
            # stage buffers: 3 rows x 128 cols = 384 bf16 elements each
            for i in range(2):
                nc.sync.dma_start(
                    out=bass_mod.AP(tensor=stage_d, offset=(i * SPAD + S) * 128,
                                    ap=[[1, (SPAD - S) * 128]]),
                    in_=zrow[0:1, 0:(SPAD - S) * 128])

            # ---------------- weights ----------------
            wxs = state.tile([76, G4], dt.float32, tag="wxs")
            wxg = state.tile([76, G4], dt.bfloat16, tag="wxg")
            for (d0, d1), (s0, s1) in COL_PERM:
                nc.gpsimd.dma_start(out=wxs[0:75, d0:d1], in_=wx_d[:, s0:s1])
            brow = state.tile([1, G4], dt.float32, tag="brow")
            for (d0, d1), (s0, s1) in COL_PERM:
                nc.gpsimd.dma_start(out=brow[0:1, d0:d1],
                                    in_=b_d[s0:s1].unsqueeze(0))
            # brow: sig cols -> 0.2*b + 0.5 ; c~ cols unchanged
            nc.vector.tensor_scalar(out=brow[:, 0:108], in0=brow[:, 0:108],
                                    scalar1=0.2, scalar2=0.5,
                                    op0=Alu.mult, op1=Alu.add)
            browb = state.tile([1, G4], dt.bfloat16, tag="browb")
            nc.vector.tensor_copy(browb[:], brow[:])
            nc.vector.tensor_scalar_mul(wxg[0:75, 0:108], wxs[0:75, 0:108], 0.2)
            nc.vector.tensor_copy(wxg[0:75, 108:144], wxs[0:75, 108:144])
            nc.sync.dma_start(out=wxg[75:76, :], in_=browb[:])

            whg = []
            whs = state.tile([108, G4], dt.float32, tag="whs")
            for gi, (dy, dx0, Kg) in enumerate(REC_GROUPS):
                r0 = dy * 180 + dx0 * 36
                wt = state.tile([Kg, G4], dt.bfloat16, name=f"whg{gi}")
                for (d0, d1), (s0, s1) in COL_PERM:
                    nc.gpsimd.dma_start(out=whs[0:Kg, d0:d1],
                                        in_=wh_d[r0:r0 + Kg, s0:s1])
                nc.vector.tensor_scalar_mul(wt[0:Kg, 0:108], whs[0:Kg, 0:108], 0.2)
                nc.vector.tensor_copy(wt[0:Kg, 108:144], whs[0:Kg, 108:144])
                whg.append(wt)

            # ---------------- im2col gather helpers ----------------
            def gather(t):
                sb = t % 2
                for dy in range(KK):
                    src = bass_mod.AP(
                        tensor=x_d,
                        offset=t * H * WC + dy * WC,
                        ap=[[4 * WC, NY], [4 * C, Ws], [1, 15]])
                    dst = bass_mod.AP(
                        tensor=stage_d,
                        offset=sb * SPAD * 128 + 15 * dy,
                        ap=[[128 * Ws, NY], [128, Ws], [1, 15]])
                    nc.gpsimd.dma_start(out=dst, in_=src)

            def transpose_x(t):
                sb = t % 2
                nc.sync.dma_start(out=X75[sb][:], in_=stage_d[sb], transpose=True)
                nc.sync.dma_start(out=X75[sb][75:76, :], in_=ones[:])

            gather(0)
            transpose_x(0)

            # ---------------- time steps ----------------
            for t in range(T):
                Xc = X75[t % 2]
                Rc = R[t % 2]
                Rn = R[(t + 1) % 2]
                last = (t == T - 1)
                if not last:
                    gather(t + 1)
                for g in range(NG):
                    ps = gpsum.tile([128, 3, G4], dt.float32, tag="gps")
                    for j in range(3):
                        blk = g * 3 + j
                        o0 = blk * 128
                        nc.tensor.matmul(ps[:, j, :], Xc[0:76, o0:o0 + 128], wxg[:],
                                         start=True, stop=(t == 0))
                        if t > 0:
                            for gi, (dy, dx0, Kg) in enumerate(REC_GROUPS):
                                off = o0 + dy * Ws + dx0
                                nc.tensor.matmul(
                                    ps[:, j, :], Rc[0:Kg, off:off + 128],
                                    whg[gi][:],
                                    start=False, stop=(gi == len(REC_GROUPS) - 1))
                    # ---- gates ----
                    z2 = zpool.tile([128, 3, 108], dt.float32, tag="z2")
                    nc.vector.tensor_scalar(out=z2[:], in0=ps[:, :, 0:108],
                                            scalar1=0.0, scalar2=1.0,
                                            op0=Alu.max, op1=Alu.min)
                    tslot = tcst[:, 3 * g:3 * g + 3, 0:36]
                    cslot = tcst[:, 3 * g:3 * g + 3, 36:72]
                    nc.scalar.activation(out=tslot, in_=ps[:, :, 108:144],
                                         func=Act.Tanh)
                    prod = ppool.tile([128, 3, 72], dt.float32, tag="prod")
                    nc.vector.tensor_tensor(out=prod[:], in0=z2[:, :, 0:72],
                                            in1=tcst[:, 3 * g:3 * g + 3, 0:72],
                                            op=Alu.mult)
                    nc.vector.tensor_tensor(out=cslot, in0=prod[:, :, 0:36],
                                            in1=prod[:, :, 36:72], op=Alu.add)
                    nc.scalar.activation(out=tslot, in_=cslot, func=Act.Tanh)
                    if last:
                        hf = hfpool.tile([128, 3, F], dt.float32, tag="hf")
                        nc.vector.tensor_tensor(out=hf[:], in0=z2[:, :, 72:108],
                                                in1=tslot, op=Alu.mult)
                        # hf flat order is (p, j, c); strip row = j*128 + p
                        nc.gpsimd.dma_start(
                            out=bass_mod.AP(tensor=hstage_d,
                                            offset=g * GSZ * F,
                                            ap=[[F, 128], [128 * F, 3], [1, F]]),
                            in_=hf[:])
                        continue
                    hsp = hpool.tile([128, 3, F], dt.bfloat16, tag="hsp")
                    nc.vector.tensor_tensor(out=hsp[:], in0=z2[:, :, 72:108],
                                            in1=tslot, op=Alu.mult)
                    # ---- transpose h back to channel-major into Rn ----
                    pt = tpsum.tile([F, GSZ], dt.bfloat16, tag="pt")
                    for j in range(3):
                        nc.tensor.transpose(pt[:, j * 128:(j + 1) * 128],
                                            hsp[:, j, :], ident[:])
                    m0 = ROFF + g * GSZ
                    m1 = min(m0 + GSZ, ROFF + S)
                    nc.scalar.copy(Rn[0:F, m0:m1], pt[:, 0:m1 - m0])
                    for a in garbage_runs_in(m0, m1):
                        nc.gpsimd.memset(
                            bass_mod.AP(tensor=Rn.tensor,
                                        offset=Rn.offset + a,
                                        ap=[[RFREE, F], [1, 4]]), 0.0)
                    for grp in (1, 2):
                        nc.sync.dma_start(
                            out=Rn[F * grp:F * (grp + 1), m0 - grp:m1 - grp],
                            in_=Rn[0:F, m0:m1])
                if not last:
                    transpose_x(t + 1)

            # ---------------- final output ----------------
            src = bass_mod.AP(tensor=hstage_d, offset=0,
                              ap=[[Ws * F, Ho], [1, Ho * F]])
            dst = bass_mod.AP(tensor=out_d, offset=0,
                              ap=[[Ho * F, Ho], [1, Ho * F]])
            nc.sync.dma_start(out=dst, in_=src)

    nc.compile()
    return nc


_CACHE = {}


def _get_nc():
    if "nc" not in _CACHE:
        import concourse.bass as bass_mod
        import concourse.bacc as bacc_mod
        import concourse.tile as tile_mod
        from concourse import mybir
        from concourse.masks import make_identity
        _CACHE["nc"] = build(bass_mod, bacc_mod, tile_mod, mybir, bass_mod,
                             make_identity)
    return _CACHE["nc"]


def run_on_hw(states_in, Wx, Wh, b, trace=False):
    from concourse.bass_utils import run_bass_kernel_spmd
    nc = _get_nc()
    wx = np.ascontiguousarray(Wx.reshape(75, G4), dtype=np.float32)
    wh = np.ascontiguousarray(Wh.reshape(900, G4), dtype=np.float32)
    bb = np.ascontiguousarray(b, dtype=np.float32)
    in_maps = [{
        "x": np.ascontiguousarray(states_in[i, :T], dtype=np.float32),
        "wx": wx, "wh": wh, "b": bb,
    } for i in range(B)]
    res = run_bass_kernel_spmd(nc, in_maps, core_ids=list(range(B)),
                               trace=trace)
    out = np.stack([res.results[i]["out"] for i in range(B)])
    return out, res


def kernel(states_in, Wx, Wh, b):
    out, _ = run_on_hw(states_in, Wx, Wh, b)
    return out.astype(np.float32)


# ---------------------------------------------------------------------------
# numpy simulation of the exact same dataflow (for offline logic validation)
# ---------------------------------------------------------------------------
def numpy_sim(states_in, Wx, Wh, b, bf16=True):
    import ml_dtypes

    def q(a):
        return a.astype(ml_dtypes.bfloat16).astype(np.float32) if bf16 else a

    out = np.zeros((B, Ho * Ho * F), np.float32)
    wx = Wx.reshape(75, G4).astype(np.float32)
    wh = Wh.reshape(900, G4).astype(np.float32)

    # permuted + scaled weights
    def permcols(a):
        r = np.zeros_like(a)
        for (d0, d1), (s0, s1) in COL_PERM:
            r[..., d0:d1] = a[..., s0:s1]
        return r

    wxp = permcols(wx)
    whp = permcols(wh)
    bp = permcols(b.astype(np.float32))
    brow = np.concatenate([0.2 * bp[0:108] + 0.5, bp[108:144]])
    wxg = np.vstack([wxp, brow[None, :]])
    wxg[0:75, 0:108] *= 0.2
    wxg = q(wxg)
    whgs = []
    for dy, dx0, Kg in REC_GROUPS:
        r0 = dy * 180 + dx0 * 36
        wt = whp[r0:r0 + Kg].copy()
        wt[:, 0:108] *= 0.2
        whgs.append(q(wt))

    ys, xos = np.divmod(np.arange(S), Ws)
    i0 = (4 * ys[:, None] + np.arange(KK)[None, :]) * WC + 4 * xos[:, None] * C
    gidx = i0[:, :, None] + np.arange(15)[None, None, :]  # [S, 5, 15]

    for bi in range(B):
        xflat = states_in[bi, :T].astype(np.float32).reshape(T, -1)
        R0 = [np.zeros((108, RFREE), np.float32) for _ in range(2)]
        cst = np.zeros((SPAD, F), np.float32)
        hfinal = np.zeros((SPAD, F), np.float32)

        def build_x75(t):
            X = np.zeros((76, SPAD), np.float32)
            X[75] = 1.0
            vals = q(xflat[t][gidx])            # [S, 5, 15]
            X[0:75, 0:S] = vals.reshape(S, 75).T
            return X

        for t in range(T):
            Xc = build_x75(t)
            Rc = R0[t % 2]
            Rn = R0[(t + 1) % 2]
            last = (t == T - 1)
            h_all = np.zeros((SPAD, F), np.float32)
            for blk in range(NBLK):
                o0 = blk * 128
                ps = Xc[:, o0:o0 + 128].T @ wxg
                if t > 0:
                    for gi, (dy, dx0, Kg) in enumerate(REC_GROUPS):
                        off = o0 + dy * Ws + dx0
                        ps = ps + Rc[0:Kg, off:off + 128].T @ whgs[gi]
                sig = np.clip(ps[:, 0:108], 0.0, 1.0)
                tta = np.tanh(ps[:, 108:144])
                i_g, f_g, o_g = sig[:, 0:36], sig[:, 36:72], sig[:, 72:108]
                c_old = cst[o0:o0 + 128]
                c_new = f_g * c_old + i_g * tta
                cst[o0:o0 + 128] = c_new
                h_all[o0:o0 + 128] = o_g * np.tanh(c_new)
            if last:
                hfinal = h_all
            else:
                Rn[:] = 0.0
                Rn[0:F, ROFF:ROFF + S] = q(h_all[:S]).T
                mask = np.ones(S, bool)
                mask[xos >= 63] = False
                Rn[0:F, ROFF:ROFF + S][:, ~mask] = 0.0
                Rn[F:2 * F, :-1] = Rn[0:F, 1:]
                Rn[2 * F:3 * F, :-2] = Rn[0:F, 2:]

        hf = hfinal[:S].reshape(NY, Ws, F)[:, 0:Ho, :]
        out[bi] = hf.reshape(-1)
    return out


# revision 17
# speedup vs baseline: 76.2137x; 1.0849x over previous
"""ConvLSTM2D (filters=36, kernel 5x5, strides 4, valid) + Flatten on 8 trn2 cores.

Data-parallel over batch (B=8): core i handles batch element i end-to-end.

Per-core dataflow (all shapes per core):
  x: (16, 256, 256, 3) f32.  Wx: (75, 144), Wh: (900, 144), b: (144) f32.
  Input conv (stride 4, VALID) -> (63, 63, 144) per step, computed as one
  K=76 GEMM per 128-spatial block from a DMA-gathered im2col (bias folded
  into a ones-row).  Recurrent conv (SAME, stride 1) -> 10 accumulating
  GEMMs per block from a replicated channel-major padded h buffer (halo
  trick).  Gates are evaluated spatial-major ([128 positions, 144 cols]),
  LSTM state c kept in fp32, h written back channel-major via PE transpose.

Strip layout: o = y*67 + xo, y in [0,63), xo in [0,67); xo>=63 are halo
(garbage) columns that map exactly onto the zero-padding columns of the
padded h image, so they are memset to zero after each step.
"""

import os
import sys
import numpy as np

sys.path.insert(0, "/opt/trn_rl_repo")
sys.path.insert(0, "/opt/pypackages")

# The kernel executes on the 8 axon-tunneled NeuronCores via PJRT. If the
# calling process pinned jax to cpu (common for running the jax reference)
# and jax has not been imported yet, drop the pin so the axon platform is
# discoverable when the bass runtime initializes jax.
if "jax" not in sys.modules and os.environ.get("JAX_PLATFORMS") == "cpu":
    del os.environ["JAX_PLATFORMS"]

B = 8
T = int(os.environ.get("KERNEL_T", "16"))
H = W = 256
C = 3
F = 36
KK = 5
G4 = 4 * F            # 144
Ho = (H - KK) // 4 + 1  # 63
Ws = Ho + 4             # 67 strip cols (63 valid + 4 halo)
NY = Ho                 # 63 strip rows
S = NY * Ws             # 4221
SPAD = 4224             # 33 blocks of 128
NBLK = SPAD // 128      # 33
NG = 11                 # gate groups of 3 blocks
GSZ = 384               # strip positions per gate group
WC = W * C              # 768
RFREE = 4608            # R buffer free size (needs >= 4224 + 272 + margin)
ROFF = 136              # h strip position s lands at R0 column s + 136

# recurrent tap groups: (dy, dx0, K) ; lhsT = R[0:K, o + dy*67 + dx0 : +128]
REC_GROUPS = []
for dy in range(KK):
    REC_GROUPS.append((dy, 0, 108))
    REC_GROUPS.append((dy, 3, 72))

# gate column permutation: psum cols = [i(0:36) f(36:72) o(72:108) c~(108:144)]
# original keras order in the 144 axis: [i f c o]
# dest col range -> src col range in (Wx, Wh, b)
COL_PERM = [((0, 72), (0, 72)), ((72, 108), (108, 144)), ((108, 144), (72, 108))]


def garbage_runs_in(m0, m1):
    """R0 column runs (start, 4) inside [m0, m1) holding halo strip cols."""
    runs = []
    y = max(0, (m0 - ROFF - 63 + 66) // 67)
    while True:
        a = ROFF + y * 67 + 63
        if a >= m1:
            break
        if a >= m0:
            assert a + 4 <= m1, "garbage run straddles group boundary"
            runs.append(a)
        y += 1
    return runs


def build(nc_mod, bacc_mod, tile_mod, mybir, bass_mod, make_identity):
    nc = bacc_mod.Bacc("TRN2", target_bir_lowering=False, debug=False,
                       enable_asserts=True, num_devices=B)
    dt = mybir.dt
    Alu = mybir.AluOpType
    Act = mybir.ActivationFunctionType

    x_d = nc.dram_tensor("x", [T, H, W, C], dt.float32, kind="ExternalInput")
    wx_d = nc.dram_tensor("wx", [75, G4], dt.float32, kind="ExternalInput")
    wh_d = nc.dram_tensor("wh", [900, G4], dt.float32, kind="ExternalInput")
    b_d = nc.dram_tensor("b", [G4], dt.float32, kind="ExternalInput")
    out_d = nc.dram_tensor("out", [Ho * Ho * F], dt.float32, kind="ExternalOutput")
    stage_d = nc.dram_tensor("stage", [2, SPAD, 128], dt.bfloat16)
    hstage_d = nc.dram_tensor("hstage", [SPAD, F], dt.float32)

    with tile_mod.TileContext(nc) as tc:
        import contextlib
        with contextlib.ExitStack() as ctx:
            state = ctx.enter_context(tc.tile_pool(name="state", bufs=1))
            zpool = ctx.enter_context(tc.tile_pool(name="z2", bufs=6))
            ppool = ctx.enter_context(tc.tile_pool(name="prod", bufs=6))
            hpool = ctx.enter_context(tc.tile_pool(name="hsp", bufs=6))
            hfpool = ctx.enter_context(tc.tile_pool(name="hspf", bufs=4))
            gpsum = ctx.enter_context(tc.tile_pool(name="gps", bufs=6, space="PSUM"))
            tpsum = ctx.enter_context(tc.tile_pool(name="tps", bufs=2, space="PSUM"))

            # ---------------- persistent state ----------------
            X75 = [state.tile([128, SPAD], dt.bfloat16, name=f"x75_{i}") for i in range(2)]
            R = [state.tile([108, RFREE], dt.bfloat16, name=f"R{i}") for i in range(2)]
            tcst = state.tile([128, NBLK, 72], dt.float32, tag="tc")
            ident = state.tile([128, 128], dt.bfloat16, tag="ident")
            ones = state.tile([1, SPAD], dt.bfloat16, tag="ones")
            zrow = state.tile([1, 384], dt.bfloat16, tag="zrow")

            # only the borders stay permanently zero; [ROFF-2, ROFF+S) is
            # fully rewritten every step by evac + replicas
            for Ri in R:
                nc.vector.memset(Ri[:, 0:ROFF], 0.0)
                nc.vector.memset(Ri[:, ROFF + S - 4:RFREE], 0.0)
            nc.vector.memset(tcst[:], 0.0)
            nc.vector.memset(ones[:], 1.0)
            nc.vector.memset(zrow[:], 0.0)
            make_identity(nc, ident[:])
            # zero the never-gathered tail rows (strip 4221..4223) of both
            # stage buffers: 3 rows x 128 cols = 384 bf16 elements each
            for i in range(2):
                nc.sync.dma_start(
                    out=bass_mod.AP(tensor=stage_d, offset=(i * SPAD + S) * 128,
                                    ap=[[1, (SPAD - S) * 128]]),
                    in_=zrow[0:1, 0:(SPAD - S) * 128])

            # ---------------- im2col gather helpers ----------------
            def gather(t):
                sb = t % 2
                for dy in range(KK):
                    src = bass_mod.AP(
                        tensor=x_d,
                        offset=t * H * WC + dy * WC,
                        ap=[[4 * WC, NY], [4 * C, Ws], [1, 15]])
                    dst = bass_mod.AP(
                        tensor=stage_d,
                        offset=sb * SPAD * 128 + 15 * dy,
                        ap=[[128 * Ws, NY], [128, Ws], [1, 15]])
                    nc.gpsimd.dma_start(out=dst, in_=src)

            def transpose_x(t):
                sb = t % 2
                nc.sync.dma_start(out=X75[sb][:], in_=stage_d[sb], transpose=True)
                nc.sync.dma_start(out=X75[sb][75:76, :], in_=ones[:])

            gather(0)
            transpose_x(0)

            # ---------------- weights ----------------
            # raw (unpermuted) loads; permutation done by slice ops below:
            # dest [0:72]=src[0:72] (i,f: x0.2), dest [72:108]=src[108:144]
            # (o: x0.2), dest [108:144]=src[72:108] (c~: copy)
            wxs = state.tile([76, G4], dt.float32, tag="wxs")
            wxg = state.tile([76, G4], dt.bfloat16, tag="wxg")
            nc.sync.dma_start(out=wxs[0:75, :], in_=wx_d[:])
            brow = state.tile([1, G4], dt.float32, tag="brow")
            nc.sync.dma_start(out=brow[0:1, :], in_=b_d[:].unsqueeze(0))
            browp = state.tile([1, G4], dt.float32, tag="browp")
            nc.vector.tensor_scalar(out=browp[:, 0:72], in0=brow[:, 0:72],
                                    scalar1=0.2, scalar2=0.5,
                                    op0=Alu.mult, op1=Alu.add)
            nc.vector.tensor_scalar(out=browp[:, 72:108], in0=brow[:, 108:144],
                                    scalar1=0.2, scalar2=0.5,
                                    op0=Alu.mult, op1=Alu.add)
            nc.vector.tensor_copy(browp[:, 108:144], brow[:, 72:108])
            browb = state.tile([1, G4], dt.bfloat16, tag="browb")
            nc.vector.tensor_copy(browb[:], browp[:])
            nc.vector.tensor_scalar_mul(wxg[0:75, 0:72], wxs[0:75, 0:72], 0.2)
            nc.vector.tensor_scalar_mul(wxg[0:75, 72:108], wxs[0:75, 108:144], 0.2)
            nc.vector.tensor_copy(wxg[0:75, 108:144], wxs[0:75, 72:108])
            nc.sync.dma_start(out=wxg[75:76, :], in_=browb[:])

            whg = []
            wstage = ctx.enter_context(tc.tile_pool(name="wstage", bufs=3))
            for gi, (dy, dx0, Kg) in enumerate(REC_GROUPS):
                r0 = dy * 180 + dx0 * 36
                wt = state.tile([Kg, G4], dt.bfloat16, name=f"whg{gi}")
                whs = wstage.tile([108, G4], dt.float32, tag="whs")
                nc.sync.dma_start(out=whs[0:Kg, :], in_=wh_d[r0:r0 + Kg, :])
                nc.vector.tensor_scalar_mul(wt[0:Kg, 0:72], whs[0:Kg, 0:72], 0.2)
                nc.vector.tensor_scalar_mul(wt[0:Kg, 72:108], whs[0:Kg, 108:144], 0.2)
                nc.vector.tensor_copy(wt[0:Kg, 108:144], whs[0:Kg, 72:108])
                whg.append(wt)

            # ---------------- time steps ----------------
            for t in range(T):
                Xc = X75[t % 2]
                Rc = R[t % 2]
                Rn = R[(t + 1) % 2]
                last = (t == T - 1)
                if not last:
                    gather(t + 1)
                    transpose_x(t + 1)
                for g in range(NG):
                    ps = gpsum.tile([128, 3, G4], dt.float32, tag="gps")
                    for j in range(3):
                        blk = g * 3 + j
                        o0 = blk * 128
                        nc.tensor.matmul(ps[:, j, :], Xc[0:76, o0:o0 + 128], wxg[:],
                                         start=True, stop=(t == 0))
                        if t > 0:
                            for gi, (dy, dx0, Kg) in enumerate(REC_GROUPS):
                                off = o0 + dy * Ws + dx0
                                nc.tensor.matmul(
                                    ps[:, j, :], Rc[0:Kg, off:off + 128],
                                    whg[gi][:],
                                    start=False, stop=(gi == len(REC_GROUPS) - 1))
                    # ---- gates ----
                    z2 = zpool.tile([128, 3, 108], dt.float32, tag="z2")
                    nc.vector.tensor_scalar(out=z2[:], in0=ps[:, :, 0:108],
                                            scalar1=0.0, scalar2=1.0,
                                            op0=Alu.max, op1=Alu.min)
                    tslot = tcst[:, 3 * g:3 * g + 3, 0:36]
                    cslot = tcst[:, 3 * g:3 * g + 3, 36:72]
                    nc.scalar.activation(out=tslot, in_=ps[:, :, 108:144],
                                         func=Act.Tanh)
                    prod = ppool.tile([128, 3, 72], dt.float32, tag="prod")
                    nc.vector.tensor_tensor(out=prod[:], in0=z2[:, :, 0:72],
                                            in1=tcst[:, 3 * g:3 * g + 3, 0:72],
                                            op=Alu.mult)
                    nc.vector.tensor_tensor(out=cslot, in0=prod[:, :, 0:36],
                                            in1=prod[:, :, 36:72], op=Alu.add)
                    nc.scalar.activation(out=tslot, in_=cslot, func=Act.Tanh)
                    if last:
                        hf = hfpool.tile([128, 3, F], dt.float32, tag="hf")
                        nc.vector.tensor_tensor(out=hf[:], in0=z2[:, :, 72:108],
                                                in1=tslot, op=Alu.mult)
                        # hf flat order is (p, j, c); strip row = j*128 + p
                        nc.gpsimd.dma_start(
                            out=bass_mod.AP(tensor=hstage_d,
                                            offset=g * GSZ * F,
                                            ap=[[F, 128], [128 * F, 3], [1, F]]),
                            in_=hf[:])
                        continue
                    hsp = hpool.tile([128, 3, F], dt.bfloat16, tag="hsp")
                    nc.vector.tensor_tensor(out=hsp[:], in0=z2[:, :, 72:108],
                                            in1=tslot, op=Alu.mult)
                    # ---- transpose h back to channel-major into Rn ----
                    pt = tpsum.tile([F, GSZ], dt.bfloat16, tag="pt")
                    for j in range(3):
                        nc.tensor.transpose(pt[:, j * 128:(j + 1) * 128],
                                            hsp[:, j, :], ident[:])
                    m0 = ROFF + g * GSZ
                    m1 = min(m0 + GSZ, ROFF + S)
                    nc.scalar.copy(Rn[0:F, m0:m1], pt[:, 0:m1 - m0])
                    for a in garbage_runs_in(m0, m1):
                        nc.gpsimd.memset(
                            bass_mod.AP(tensor=Rn.tensor,
                                        offset=Rn.offset + a,
                                        ap=[[RFREE, F], [1, 4]]), 0.0)
                    for grp in (1, 2):
                        nc.sync.dma_start(
                            out=Rn[F * grp:F * (grp + 1), m0 - grp:m1 - grp],
                            in_=Rn[0:F, m0:m1])

            # ---------------- final output ----------------
            src = bass_mod.AP(tensor=hstage_d, offset=0,
                              ap=[[Ws * F, Ho], [1, Ho * F]])
            dst = bass_mod.AP(tensor=out_d, offset=0,
                              ap=[[Ho * F, Ho], [1, Ho * F]])
            nc.sync.dma_start(out=dst, in_=src)

    nc.compile()
    return nc


_CACHE = {}


def _get_nc():
    if "nc" not in _CACHE:
        import concourse.bass as bass_mod
        import concourse.bacc as bacc_mod
        import concourse.tile as tile_mod
        from concourse import mybir
        from concourse.masks import make_identity
        _CACHE["nc"] = build(bass_mod, bacc_mod, tile_mod, mybir, bass_mod,
                             make_identity)
    return _CACHE["nc"]


def run_on_hw(states_in, Wx, Wh, b, trace=False):
    from concourse.bass_utils import run_bass_kernel_spmd
    nc = _get_nc()
    wx = np.ascontiguousarray(Wx.reshape(75, G4), dtype=np.float32)
    wh = np.ascontiguousarray(Wh.reshape(900, G4), dtype=np.float32)
    bb = np.ascontiguousarray(b, dtype=np.float32)
    in_maps = [{
        "x": np.ascontiguousarray(states_in[i, :T], dtype=np.float32),
        "wx": wx, "wh": wh, "b": bb,
    } for i in range(B)]
    res = run_bass_kernel_spmd(nc, in_maps, core_ids=list(range(B)),
                               trace=trace)
    out = np.stack([res.results[i]["out"] for i in range(B)])
    return out, res


def kernel(states_in, Wx, Wh, b):
    out, _ = run_on_hw(states_in, Wx, Wh, b)
    return out.astype(np.float32)


# ---------------------------------------------------------------------------
# numpy simulation of the exact same dataflow (for offline logic validation)
# ---------------------------------------------------------------------------
def numpy_sim(states_in, Wx, Wh, b, bf16=True):
    import ml_dtypes

    def q(a):
        return a.astype(ml_dtypes.bfloat16).astype(np.float32) if bf16 else a

    out = np.zeros((B, Ho * Ho * F), np.float32)
    wx = Wx.reshape(75, G4).astype(np.float32)
    wh = Wh.reshape(900, G4).astype(np.float32)

    # permuted + scaled weights
    def permcols(a):
        r = np.zeros_like(a)
        for (d0, d1), (s0, s1) in COL_PERM:
            r[..., d0:d1] = a[..., s0:s1]
        return r

    wxp = permcols(wx)
    whp = permcols(wh)
    bp = permcols(b.astype(np.float32))
    brow = np.concatenate([0.2 * bp[0:108] + 0.5, bp[108:144]])
    wxg = np.vstack([wxp, brow[None, :]])
    wxg[0:75, 0:108] *= 0.2
    wxg = q(wxg)
    whgs = []
    for dy, dx0, Kg in REC_GROUPS:
        r0 = dy * 180 + dx0 * 36
        wt = whp[r0:r0 + Kg].copy()
        wt[:, 0:108] *= 0.2
        whgs.append(q(wt))

    ys, xos = np.divmod(np.arange(S), Ws)
    i0 = (4 * ys[:, None] + np.arange(KK)[None, :]) * WC + 4 * xos[:, None] * C
    gidx = i0[:, :, None] + np.arange(15)[None, None, :]  # [S, 5, 15]

    for bi in range(B):
        xflat = states_in[bi, :T].astype(np.float32).reshape(T, -1)
        R0 = [np.zeros((108, RFREE), np.float32) for _ in range(2)]
        cst = np.zeros((SPAD, F), np.float32)
        hfinal = np.zeros((SPAD, F), np.float32)

        def build_x75(t):
            X = np.zeros((76, SPAD), np.float32)
            X[75] = 1.0
            vals = q(xflat[t][gidx])            # [S, 5, 15]
            X[0:75, 0:S] = vals.reshape(S, 75).T
            return X

        for t in range(T):
            Xc = build_x75(t)
            Rc = R0[t % 2]
            Rn = R0[(t + 1) % 2]
            last = (t == T - 1)
            h_all = np.zeros((SPAD, F), np.float32)
            for blk in range(NBLK):
                o0 = blk * 128
                ps = Xc[:, o0:o0 + 128].T @ wxg
                if t > 0:
                    for gi, (dy, dx0, Kg) in enumerate(REC_GROUPS):
                        off = o0 + dy * Ws + dx0
                        ps = ps + Rc[0:Kg, off:off + 128].T @ whgs[gi]
                sig = np.clip(ps[:, 0:108], 0.0, 1.0)
                tta = np.tanh(ps[:, 108:144])
                i_g, f_g, o_g = sig[:, 0:36], sig[:, 36:72], sig[:, 72:108]
                c_old = cst[o0:o0 + 128]
                c_new = f_g * c_old + i_g * tta
                cst[o0:o0 + 128] = c_new
                h_all[o0:o0 + 128] = o_g * np.tanh(c_new)
            if last:
                hfinal = h_all
            else:
                Rn[:] = 0.0
                Rn[0:F, ROFF:ROFF + S] = q(h_all[:S]).T
                mask = np.ones(S, bool)
                mask[xos >= 63] = False
                Rn[0:F, ROFF:ROFF + S][:, ~mask] = 0.0
                Rn[F:2 * F, :-1] = Rn[0:F, 1:]
                Rn[2 * F:3 * F, :-2] = Rn[0:F, 2:]

        hf = hfinal[:S].reshape(NY, Ws, F)[:, 0:Ho, :]
        out[bi] = hf.reshape(-1)
    return out


# revision 21
# speedup vs baseline: 83.6997x; 1.0982x over previous
"""ConvLSTM2D (filters=36, kernel 5x5, strides 4, valid) + Flatten on 8 trn2 cores.

Data-parallel over batch (B=8): core i handles batch element i end-to-end.

Per-core dataflow (all shapes per core):
  x: (16, 256, 256, 3) f32.  Wx: (75, 144), Wh: (900, 144), b: (144) f32.
  Input conv (stride 4, VALID) -> (63, 63, 144) per step, computed as one
  K=76 GEMM per 128-spatial block from a DMA-gathered im2col (bias folded
  into a ones-row).  Recurrent conv (SAME, stride 1) -> 10 accumulating
  GEMMs per block from a replicated channel-major padded h buffer (halo
  trick).  Gates are evaluated spatial-major ([128 positions, 144 cols]),
  LSTM state c kept in fp32, h written back channel-major via PE transpose.

Strip layout: o = y*67 + xo, y in [0,63), xo in [0,67); xo>=63 are halo
(garbage) columns that map exactly onto the zero-padding columns of the
padded h image, so they are memset to zero after each step.
"""

import os
import sys
import numpy as np

sys.path.insert(0, "/opt/trn_rl_repo")
sys.path.insert(0, "/opt/pypackages")

# The kernel executes on the 8 axon-tunneled NeuronCores via PJRT. If the
# calling process pinned jax to cpu (common for running the jax reference)
# and jax has not been imported yet, drop the pin so the axon platform is
# discoverable when the bass runtime initializes jax.
if "jax" not in sys.modules and os.environ.get("JAX_PLATFORMS") == "cpu":
    del os.environ["JAX_PLATFORMS"]

B = 8
T = int(os.environ.get("KERNEL_T", "16"))
H = W = 256
C = 3
F = 36
KK = 5
G4 = 4 * F            # 144
Ho = (H - KK) // 4 + 1  # 63
Ws = Ho + 4             # 67 strip cols (63 valid + 4 halo)
NY = Ho                 # 63 strip rows
S = NY * Ws             # 4221
SPAD = 4224             # 33 blocks of 128
NBLK = SPAD // 128      # 33
NG = 11                 # gate groups of 3 blocks
GSZ = 384               # strip positions per gate group
WC = W * C              # 768
RFREE = 4608            # R buffer free size (needs >= 4224 + 272 + margin)
ROFF = 136              # h strip position s lands at R0 column s + 136

# recurrent tap groups: (dy, dx0, K) ; lhsT = R[0:K, o + dy*67 + dx0 : +128]
REC_GROUPS = []
for dy in range(KK):
    REC_GROUPS.append((dy, 0, 108))
    REC_GROUPS.append((dy, 3, 72))

# gate column permutation: psum cols = [i(0:36) f(36:72) o(72:108) c~(108:144)]
# original keras order in the 144 axis: [i f c o]
# dest col range -> src col range in (Wx, Wh, b)
COL_PERM = [((0, 72), (0, 72)), ((72, 108), (108, 144)), ((108, 144), (72, 108))]


def garbage_runs_in(m0, m1):
    """R0 column runs (start, 4) inside [m0, m1) holding halo strip cols."""
    runs = []
    y = max(0, (m0 - ROFF - 63 + 66) // 67)
    while True:
        a = ROFF + y * 67 + 63
        if a >= m1:
            break
        if a >= m0:
            assert a + 4 <= m1, "garbage run straddles group boundary"
            runs.append(a)
        y += 1
    return runs


def build(nc_mod, bacc_mod, tile_mod, mybir, bass_mod, make_identity):
    nc = bacc_mod.Bacc("TRN2", target_bir_lowering=False, debug=False,
                       enable_asserts=True, num_devices=B)
    dt = mybir.dt
    Alu = mybir.AluOpType
    Act = mybir.ActivationFunctionType

    x_d = nc.dram_tensor("x", [T, H, W, C], dt.float32, kind="ExternalInput")
    wx_d = nc.dram_tensor("wx", [75, G4], dt.float32, kind="ExternalInput")
    wh_d = nc.dram_tensor("wh", [900, G4], dt.float32, kind="ExternalInput")
    b_d = nc.dram_tensor("b", [G4], dt.float32, kind="ExternalInput")
    out_d = nc.dram_tensor("out", [Ho * Ho * F], dt.float32, kind="ExternalOutput")
    stage_d = nc.dram_tensor("stage", [2, SPAD, 128], dt.bfloat16)
    hstage_d = nc.dram_tensor("hstage", [SPAD, F], dt.float32)

    with tile_mod.TileContext(nc) as tc:
        import contextlib
        with contextlib.ExitStack() as ctx:
            state = ctx.enter_context(tc.tile_pool(name="state", bufs=1))
            zpool = ctx.enter_context(tc.tile_pool(name="z2", bufs=6))
            ppool = ctx.enter_context(tc.tile_pool(name="prod", bufs=6))
            hpool = ctx.enter_context(tc.tile_pool(name="hsp", bufs=6))
            hfpool = ctx.enter_context(tc.tile_pool(name="hspf", bufs=4))
            gpsum = ctx.enter_context(tc.tile_pool(name="gps", bufs=6, space="PSUM"))
            tpsum = ctx.enter_context(tc.tile_pool(name="tps", bufs=2, space="PSUM"))

            # ---------------- persistent state ----------------
            X75 = [state.tile([128, SPAD], dt.bfloat16, name=f"x75_{i}") for i in range(2)]
            R = [state.tile([108, RFREE], dt.bfloat16, name=f"R{i}") for i in range(2)]
            tcst = state.tile([128, NBLK, 72], dt.float32, tag="tc")
            ident = state.tile([128, 128], dt.bfloat16, tag="ident")
            ones = state.tile([1, SPAD], dt.bfloat16, tag="ones")
            zrow = state.tile([1, 384], dt.bfloat16, tag="zrow")

            # only the borders stay permanently zero; [ROFF-2, ROFF+S) is
            # fully rewritten every step by evac + replicas
            for Ri in R:
                nc.vector.memset(Ri[:, 0:ROFF], 0.0)
                nc.vector.memset(Ri[:, ROFF + S - 4:RFREE], 0.0)
            nc.vector.memset(tcst[:], 0.0)
            nc.vector.memset(ones[:], 1.0)
            nc.vector.memset(zrow[:], 0.0)
            make_identity(nc, ident[:])
            # zero the never-gathered tail rows (strip 4221..4223) of both
            # stage buffers: 3 rows x 128 cols = 384 bf16 elements each
            for i in range(2):
                nc.sync.dma_start(
                    out=bass_mod.AP(tensor=stage_d, offset=(i * SPAD + S) * 128,
                                    ap=[[1, (SPAD - S) * 128]]),
                    in_=zrow[0:1, 0:(SPAD - S) * 128])

            # ---------------- im2col gather helpers ----------------
            def gather(t):
                sb = t % 2
                for dy in range(KK):
                    src = bass_mod.AP(
                        tensor=x_d,
                        offset=t * H * WC + dy * WC,
                        ap=[[4 * WC, NY], [4 * C, Ws], [1, 15]])
                    dst = bass_mod.AP(
                        tensor=stage_d,
                        offset=sb * SPAD * 128 + 15 * dy,
                        ap=[[128 * Ws, NY], [128, Ws], [1, 15]])
                    nc.gpsimd.dma_start(out=dst, in_=src)

            def transpose_x(t):
                sb = t % 2
                nc.sync.dma_start(out=X75[sb][:], in_=stage_d[sb], transpose=True)
                nc.sync.dma_start(out=X75[sb][75:76, :], in_=ones[:])

            gather(0)
            transpose_x(0)

            # ---------------- weights ----------------
            # raw (unpermuted) loads; permutation done by slice ops below:
            # dest [0:72]=src[0:72] (i,f: x0.2), dest [72:108]=src[108:144]
            # (o: x0.2), dest [108:144]=src[72:108] (c~: copy)
            wxs = state.tile([76, G4], dt.float32, tag="wxs")
            wxg = state.tile([76, G4], dt.bfloat16, tag="wxg")
            nc.sync.dma_start(out=wxs[0:75, :], in_=wx_d[:])
            brow = state.tile([1, G4], dt.float32, tag="brow")
            nc.sync.dma_start(out=brow[0:1, :], in_=b_d[:].unsqueeze(0))
            browp = state.tile([1, G4], dt.float32, tag="browp")
            nc.vector.tensor_scalar(out=browp[:, 0:72], in0=brow[:, 0:72],
                                    scalar1=0.2, scalar2=0.5,
                                    op0=Alu.mult, op1=Alu.add)
            nc.vector.tensor_scalar(out=browp[:, 72:108], in0=brow[:, 108:144],
                                    scalar1=0.2, scalar2=0.5,
                                    op0=Alu.mult, op1=Alu.add)
            nc.vector.tensor_copy(browp[:, 108:144], brow[:, 72:108])
            browb = state.tile([1, G4], dt.bfloat16, tag="browb")
            nc.vector.tensor_copy(browb[:], browp[:])
            nc.vector.tensor_scalar_mul(wxg[0:75, 0:72], wxs[0:75, 0:72], 0.2)
            nc.vector.tensor_scalar_mul(wxg[0:75, 72:108], wxs[0:75, 108:144], 0.2)
            nc.vector.tensor_copy(wxg[0:75, 108:144], wxs[0:75, 72:108])
            nc.sync.dma_start(out=wxg[75:76, :], in_=browb[:])

            whg = []
            wstage = ctx.enter_context(tc.tile_pool(name="wstage", bufs=3))
            for gi, (dy, dx0, Kg) in enumerate(REC_GROUPS):
                r0 = dy * 180 + dx0 * 36
                wt = state.tile([Kg, G4], dt.bfloat16, name=f"whg{gi}")
                whs = wstage.tile([108, G4], dt.float32, tag="whs")
                nc.sync.dma_start(out=whs[0:Kg, :], in_=wh_d[r0:r0 + Kg, :])
                nc.gpsimd.tensor_scalar_mul(wt[0:Kg, 0:72], whs[0:Kg, 0:72], 0.2)
                nc.gpsimd.tensor_scalar_mul(wt[0:Kg, 72:108], whs[0:Kg, 108:144], 0.2)
                nc.gpsimd.tensor_copy(wt[0:Kg, 108:144], whs[0:Kg, 72:108])
                whg.append(wt)

            # ---------------- time steps ----------------
            for t in range(T):
                Xc = X75[t % 2]
                Rc = R[t % 2]
                Rn = R[(t + 1) % 2]
                last = (t == T - 1)
                if not last:
                    gather(t + 1)
                    transpose_x(t + 1)
                for g in range(NG):
                    ps = gpsum.tile([128, 3, G4], dt.float32, tag="gps")
                    for j in range(3):
                        blk = g * 3 + j
                        o0 = blk * 128
                        nc.tensor.matmul(ps[:, j, :], Xc[0:76, o0:o0 + 128], wxg[:],
                                         start=True, stop=(t == 0))
                        if t > 0:
                            for gi, (dy, dx0, Kg) in enumerate(REC_GROUPS):
                                off = o0 + dy * Ws + dx0
                                nc.tensor.matmul(
                                    ps[:, j, :], Rc[0:Kg, off:off + 128],
                                    whg[gi][:],
                                    start=False, stop=(gi == len(REC_GROUPS) - 1))
                    # ---- gates ----
                    z2 = zpool.tile([128, 3, 108], dt.float32, tag="z2")
                    nc.vector.tensor_scalar(out=z2[:], in0=ps[:, :, 0:108],
                                            scalar1=0.0, scalar2=1.0,
                                            op0=Alu.max, op1=Alu.min)
                    tslot = tcst[:, 3 * g:3 * g + 3, 0:36]
                    cslot = tcst[:, 3 * g:3 * g + 3, 36:72]
                    nc.scalar.activation(out=tslot, in_=ps[:, :, 108:144],
                                         func=Act.Tanh)
                    prod = ppool.tile([128, 3, 72], dt.float32, tag="prod")
                    nc.vector.tensor_tensor(out=prod[:], in0=z2[:, :, 0:72],
                                            in1=tcst[:, 3 * g:3 * g + 3, 0:72],
                                            op=Alu.mult)
                    nc.vector.tensor_tensor(out=cslot, in0=prod[:, :, 0:36],
                                            in1=prod[:, :, 36:72], op=Alu.add)
                    nc.scalar.activation(out=tslot, in_=cslot, func=Act.Tanh)
                    if last:
                        hf = hfpool.tile([128, 3, F], dt.float32, tag="hf")
                        nc.vector.tensor_tensor(out=hf[:], in0=z2[:, :, 72:108],
                                                in1=tslot, op=Alu.mult)
                        # hf flat order is (p, j, c); strip row = j*128 + p
                        nc.gpsimd.dma_start(
                            out=bass_mod.AP(tensor=hstage_d,
                                            offset=g * GSZ * F,
                                            ap=[[F, 128], [128 * F, 3], [1, F]]),
                            in_=hf[:])
                        continue
                    hsp = hpool.tile([128, 3, F], dt.bfloat16, tag="hsp")
                    nc.vector.tensor_tensor(out=hsp[:], in0=z2[:, :, 72:108],
                                            in1=tslot, op=Alu.mult)
                    # ---- transpose h back to channel-major into Rn ----
                    pt = tpsum.tile([F, GSZ], dt.bfloat16, tag="pt")
                    for j in range(3):
                        nc.tensor.transpose(pt[:, j * 128:(j + 1) * 128],
                                            hsp[:, j, :], ident[:])
                    m0 = ROFF + g * GSZ
                    m1 = min(m0 + GSZ, ROFF + S)
                    nc.scalar.copy(Rn[0:F, m0:m1], pt[:, 0:m1 - m0])
                    runs = garbage_runs_in(m0, m1)
                    if runs:
                        nc.vector.memset(
                            bass_mod.AP(tensor=Rn.tensor,
                                        offset=Rn.offset + runs[0],
                                        ap=[[RFREE, F], [67, len(runs)], [1, 4]]),
                            0.0)
                    for grp in (1, 2):
                        nc.sync.dma_start(
                            out=Rn[F * grp:F * (grp + 1), m0 - grp:m1 - grp],
                            in_=Rn[0:F, m0:m1])

            # ---------------- final output (chunked for overlap) ---------
            YCHUNK = 16
            for y0 in range(0, Ho, YCHUNK):
                ny = min(YCHUNK, Ho - y0)
                src = bass_mod.AP(tensor=hstage_d, offset=y0 * Ws * F,
                                  ap=[[Ws * F, ny], [1, Ho * F]])
                dst = bass_mod.AP(tensor=out_d, offset=y0 * Ho * F,
                                  ap=[[Ho * F, ny], [1, Ho * F]])
                nc.sync.dma_start(out=dst, in_=src)

    nc.compile()
    return nc


_CACHE = {}


def _get_nc():
    if "nc" not in _CACHE:
        import concourse.bass as bass_mod
        import concourse.bacc as bacc_mod
        import concourse.tile as tile_mod
        from concourse import mybir
        from concourse.masks import make_identity
        _CACHE["nc"] = build(bass_mod, bacc_mod, tile_mod, mybir, bass_mod,
                             make_identity)
    return _CACHE["nc"]


def run_on_hw(states_in, Wx, Wh, b, trace=False):
    from concourse.bass_utils import run_bass_kernel_spmd
    nc = _get_nc()
    wx = np.ascontiguousarray(Wx.reshape(75, G4), dtype=np.float32)
    wh = np.ascontiguousarray(Wh.reshape(900, G4), dtype=np.float32)
    bb = np.ascontiguousarray(b, dtype=np.float32)
    in_maps = [{
        "x": np.ascontiguousarray(states_in[i, :T], dtype=np.float32),
        "wx": wx, "wh": wh, "b": bb,
    } for i in range(B)]
    res = run_bass_kernel_spmd(nc, in_maps, core_ids=list(range(B)),
                               trace=trace)
    out = np.stack([res.results[i]["out"] for i in range(B)])
    return out, res


def kernel(states_in, Wx, Wh, b):
    out, _ = run_on_hw(states_in, Wx, Wh, b)
    return out.astype(np.float32)


# ---------------------------------------------------------------------------
# numpy simulation of the exact same dataflow (for offline logic validation)
# ---------------------------------------------------------------------------
def numpy_sim(states_in, Wx, Wh, b, bf16=True):
    import ml_dtypes

    def q(a):
        return a.astype(ml_dtypes.bfloat16).astype(np.float32) if bf16 else a

    out = np.zeros((B, Ho * Ho * F), np.float32)
    wx = Wx.reshape(75, G4).astype(np.float32)
    wh = Wh.reshape(900, G4).astype(np.float32)

    # permuted + scaled weights
    def permcols(a):
        r = np.zeros_like(a)
        for (d0, d1), (s0, s1) in COL_PERM:
            r[..., d0:d1] = a[..., s0:s1]
        return r

    wxp = permcols(wx)
    whp = permcols(wh)
    bp = permcols(b.astype(np.float32))
    brow = np.concatenate([0.2 * bp[0:108] + 0.5, bp[108:144]])
    wxg = np.vstack([wxp, brow[None, :]])
    wxg[0:75, 0:108] *= 0.2
    wxg = q(wxg)
    whgs = []
    for dy, dx0, Kg in REC_GROUPS:
        r0 = dy * 180 + dx0 * 36
        wt = whp[r0:r0 + Kg].copy()
        wt[:, 0:108] *= 0.2
        whgs.append(q(wt))

    ys, xos = np.divmod(np.arange(S), Ws)
    i0 = (4 * ys[:, None] + np.arange(KK)[None, :]) * WC + 4 * xos[:, None] * C
    gidx = i0[:, :, None] + np.arange(15)[None, None, :]  # [S, 5, 15]

    for bi in range(B):
        xflat = states_in[bi, :T].astype(np.float32).reshape(T, -1)
        R0 = [np.zeros((108, RFREE), np.float32) for _ in range(2)]
        cst = np.zeros((SPAD, F), np.float32)
        hfinal = np.zeros((SPAD, F), np.float32)

        def build_x75(t):
            X = np.zeros((76, SPAD), np.float32)
            X[75] = 1.0
            vals = q(xflat[t][gidx])            # [S, 5, 15]
            X[0:75, 0:S] = vals.reshape(S, 75).T
            return X

        for t in range(T):
            Xc = build_x75(t)
            Rc = R0[t % 2]
            Rn = R0[(t + 1) % 2]
            last = (t == T - 1)
            h_all = np.zeros((SPAD, F), np.float32)
            for blk in range(NBLK):
                o0 = blk * 128
                ps = Xc[:, o0:o0 + 128].T @ wxg
                if t > 0:
                    for gi, (dy, dx0, Kg) in enumerate(REC_GROUPS):
                        off = o0 + dy * Ws + dx0
                        ps = ps + Rc[0:Kg, off:off + 128].T @ whgs[gi]
                sig = np.clip(ps[:, 0:108], 0.0, 1.0)
                tta = np.tanh(ps[:, 108:144])
                i_g, f_g, o_g = sig[:, 0:36], sig[:, 36:72], sig[:, 72:108]
                c_old = cst[o0:o0 + 128]
                c_new = f_g * c_old + i_g * tta
                cst[o0:o0 + 128] = c_new
                h_all[o0:o0 + 128] = o_g * np.tanh(c_new)
            if last:
                hfinal = h_all
            else:
                Rn[:] = 0.0
                Rn[0:F, ROFF:ROFF + S] = q(h_all[:S]).T
                mask = np.ones(S, bool)
                mask[xos >= 63] = False
                Rn[0:F, ROFF:ROFF + S][:, ~mask] = 0.0
                Rn[F:2 * F, :-1] = Rn[0:F, 1:]
                Rn[2 * F:3 * F, :-2] = Rn[0:F, 2:]

        hf = hfinal[:S].reshape(NY, Ws, F)[:, 0:Ho, :]
        out[bi] = hf.reshape(-1)
    return out


# revision 24
# speedup vs baseline: 83.9522x; 1.0030x over previous
"""ConvLSTM2D (filters=36, kernel 5x5, strides 4, valid) + Flatten on 8 trn2 cores.

Data-parallel over batch (B=8): core i handles batch element i end-to-end.

Per-core dataflow (all shapes per core):
  x: (16, 256, 256, 3) f32.  Wx: (75, 144), Wh: (900, 144), b: (144) f32.
  Input conv (stride 4, VALID) -> (63, 63, 144) per step, computed as one
  K=76 GEMM per 128-spatial block from a DMA-gathered im2col (bias folded
  into a ones-row).  Recurrent conv (SAME, stride 1) -> 10 accumulating
  GEMMs per block from a replicated channel-major padded h buffer (halo
  trick).  Gates are evaluated spatial-major ([128 positions, 144 cols]),
  LSTM state c kept in fp32, h written back channel-major via PE transpose.

Strip layout: o = y*67 + xo, y in [0,63), xo in [0,67); xo>=63 are halo
(garbage) columns that map exactly onto the zero-padding columns of the
padded h image, so they are memset to zero after each step.
"""

import os
import sys
import numpy as np

sys.path.insert(0, "/opt/trn_rl_repo")
sys.path.insert(0, "/opt/pypackages")

# The kernel executes on the 8 axon-tunneled NeuronCores via PJRT. If the
# calling process pinned jax to cpu (common for running the jax reference)
# and jax has not been imported yet, drop the pin so the axon platform is
# discoverable when the bass runtime initializes jax.
if "jax" not in sys.modules and os.environ.get("JAX_PLATFORMS") == "cpu":
    del os.environ["JAX_PLATFORMS"]

B = 8
T = int(os.environ.get("KERNEL_T", "16"))
H = W = 256
C = 3
F = 36
KK = 5
G4 = 4 * F            # 144
Ho = (H - KK) // 4 + 1  # 63
Ws = Ho + 4             # 67 strip cols (63 valid + 4 halo)
NY = Ho                 # 63 strip rows
S = NY * Ws             # 4221
SPAD = 4224             # 33 blocks of 128
NBLK = SPAD // 128      # 33
NG = 11                 # gate groups of 3 blocks
GSZ = 384               # strip positions per gate group
WC = W * C              # 768
RFREE = 4608            # R buffer free size (needs >= 4224 + 272 + margin)
ROFF = 136              # h strip position s lands at R0 column s + 136

# recurrent tap groups: (dy, dx0, K) ; lhsT = R[0:K, o + dy*67 + dx0 : +128]
REC_GROUPS = []
for dy in range(KK):
    REC_GROUPS.append((dy, 0, 108))
    REC_GROUPS.append((dy, 3, 72))

# gate column permutation: psum cols = [i(0:36) f(36:72) o(72:108) c~(108:144)]
# original keras order in the 144 axis: [i f c o]
# dest col range -> src col range in (Wx, Wh, b)
COL_PERM = [((0, 72), (0, 72)), ((72, 108), (108, 144)), ((108, 144), (72, 108))]


def garbage_runs_in(m0, m1):
    """R0 column runs (start, 4) inside [m0, m1) holding halo strip cols."""
    runs = []
    y = max(0, (m0 - ROFF - 63 + 66) // 67)
    while True:
        a = ROFF + y * 67 + 63
        if a >= m1:
            break
        if a >= m0:
            assert a + 4 <= m1, "garbage run straddles group boundary"
            runs.append(a)
        y += 1
    return runs


def build(nc_mod, bacc_mod, tile_mod, mybir, bass_mod, make_identity):
    nc = bacc_mod.Bacc("TRN2", target_bir_lowering=False, debug=False,
                       enable_asserts=True, num_devices=B)
    dt = mybir.dt
    Alu = mybir.AluOpType
    Act = mybir.ActivationFunctionType

    x_d = nc.dram_tensor("x", [T, H, W, C], dt.float32, kind="ExternalInput")
    wx_d = nc.dram_tensor("wx", [75, G4], dt.float32, kind="ExternalInput")
    wh_d = nc.dram_tensor("wh", [900, G4], dt.float32, kind="ExternalInput")
    b_d = nc.dram_tensor("b", [G4], dt.float32, kind="ExternalInput")
    out_d = nc.dram_tensor("out", [Ho * Ho * F], dt.float32, kind="ExternalOutput")
    stage_d = nc.dram_tensor("stage", [2, SPAD, 128], dt.bfloat16)
    hstage_d = nc.dram_tensor("hstage", [SPAD, F], dt.float32)

    with tile_mod.TileContext(nc) as tc:
        import contextlib
        with contextlib.ExitStack() as ctx:
            state = ctx.enter_context(tc.tile_pool(name="state", bufs=1))
            zpool = ctx.enter_context(tc.tile_pool(name="z2", bufs=6))
            ppool = ctx.enter_context(tc.tile_pool(name="prod", bufs=6))
            hpool = ctx.enter_context(tc.tile_pool(name="hsp", bufs=6))
            hfpool = ctx.enter_context(tc.tile_pool(name="hspf", bufs=4))
            gpsum = ctx.enter_context(tc.tile_pool(name="gps", bufs=6, space="PSUM"))
            tpsum = ctx.enter_context(tc.tile_pool(name="tps", bufs=2, space="PSUM"))

            # ---------------- persistent state ----------------
            X75 = [state.tile([128, SPAD], dt.bfloat16, name=f"x75_{i}") for i in range(2)]
            R = [state.tile([108, RFREE], dt.bfloat16, name=f"R{i}") for i in range(2)]
            tcst = state.tile([128, NBLK, 72], dt.float32, tag="tc")
            ident = state.tile([128, 128], dt.bfloat16, tag="ident")
            ones = state.tile([1, SPAD], dt.bfloat16, tag="ones")
            zrow = state.tile([1, 384], dt.bfloat16, tag="zrow")

            # only the borders stay permanently zero; [ROFF-2, ROFF+S) is
            # fully rewritten every step by evac + replicas
            for Ri in R:
                nc.vector.memset(Ri[:, 0:ROFF], 0.0)
                nc.vector.memset(Ri[:, ROFF + S - 4:RFREE], 0.0)
            nc.vector.memset(tcst[:], 0.0)
            nc.vector.memset(ones[:], 1.0)
            nc.vector.memset(zrow[:], 0.0)
            make_identity(nc, ident[:])
            # zero the never-gathered tail rows (strip 4221..4223) of both
            # stage buffers: 3 rows x 128 cols = 384 bf16 elements each
            for i in range(2):
                nc.sync.dma_start(
                    out=bass_mod.AP(tensor=stage_d, offset=(i * SPAD + S) * 128,
                                    ap=[[1, (SPAD - S) * 128]]),
                    in_=zrow[0:1, 0:(SPAD - S) * 128])

            # ---------------- im2col gather helpers ----------------
            def gather(t):
                sb = t % 2
                for dy in range(KK):
                    src = bass_mod.AP(
                        tensor=x_d,
                        offset=t * H * WC + dy * WC,
                        ap=[[4 * WC, NY], [4 * C, Ws], [1, 15]])
                    dst = bass_mod.AP(
                        tensor=stage_d,
                        offset=sb * SPAD * 128 + 15 * dy,
                        ap=[[128 * Ws, NY], [128, Ws], [1, 15]])
                    nc.gpsimd.dma_start(out=dst, in_=src)

            def transpose_x(t):
                sb = t % 2
                nc.sync.dma_start(out=X75[sb][:], in_=stage_d[sb], transpose=True)
                nc.sync.dma_start(out=X75[sb][75:76, :], in_=ones[:])

            gather(0)
            transpose_x(0)

            # ---------------- weights ----------------
            # raw (unpermuted) loads; permutation done by slice ops below:
            # dest [0:72]=src[0:72] (i,f: x0.2), dest [72:108]=src[108:144]
            # (o: x0.2), dest [108:144]=src[72:108] (c~: copy)
            wxs = state.tile([76, G4], dt.float32, tag="wxs")
            wxg = state.tile([76, G4], dt.bfloat16, tag="wxg")
            nc.sync.dma_start(out=wxs[0:75, :], in_=wx_d[:])
            brow = state.tile([1, G4], dt.float32, tag="brow")
            nc.sync.dma_start(out=brow[0:1, :], in_=b_d[:].unsqueeze(0))
            browp = state.tile([1, G4], dt.float32, tag="browp")
            nc.vector.tensor_scalar(out=browp[:, 0:72], in0=brow[:, 0:72],
                                    scalar1=0.2, scalar2=0.5,
                                    op0=Alu.mult, op1=Alu.add)
            nc.vector.tensor_scalar(out=browp[:, 72:108], in0=brow[:, 108:144],
                                    scalar1=0.2, scalar2=0.5,
                                    op0=Alu.mult, op1=Alu.add)
            nc.vector.tensor_copy(browp[:, 108:144], brow[:, 72:108])
            browb = state.tile([1, G4], dt.bfloat16, tag="browb")
            nc.vector.tensor_copy(browb[:], browp[:])
            nc.vector.tensor_scalar_mul(wxg[0:75, 0:72], wxs[0:75, 0:72], 0.2)
            nc.vector.tensor_scalar_mul(wxg[0:75, 72:108], wxs[0:75, 108:144], 0.2)
            nc.vector.tensor_copy(wxg[0:75, 108:144], wxs[0:75, 72:108])
            nc.sync.dma_start(out=wxg[75:76, :], in_=browb[:])

            whg = []
            wstage = ctx.enter_context(tc.tile_pool(name="wstage", bufs=3))
            for gi, (dy, dx0, Kg) in enumerate(REC_GROUPS):
                r0 = dy * 180 + dx0 * 36
                wt = state.tile([Kg, G4], dt.bfloat16, name=f"whg{gi}")
                whs = wstage.tile([108, G4], dt.float32, tag="whs")
                nc.sync.dma_start(out=whs[0:Kg, :], in_=wh_d[r0:r0 + Kg, :])
                nc.vector.tensor_scalar_mul(wt[0:Kg, 0:72], whs[0:Kg, 0:72], 0.2)
                nc.vector.tensor_scalar_mul(wt[0:Kg, 72:108], whs[0:Kg, 108:144], 0.2)
                nc.vector.tensor_copy(wt[0:Kg, 108:144], whs[0:Kg, 72:108])
                whg.append(wt)

            # ---------------- time steps ----------------
            for t in range(T):
                Xc = X75[t % 2]
                Rc = R[t % 2]
                Rn = R[(t + 1) % 2]
                last = (t == T - 1)
                if not last:
                    gather(t + 1)
                    transpose_x(t + 1)
                for g in range(NG):
                    ps = gpsum.tile([128, 3, G4], dt.float32, tag="gps")
                    for j in range(3):
                        blk = g * 3 + j
                        o0 = blk * 128
                        nc.tensor.matmul(ps[:, j, :], Xc[0:76, o0:o0 + 128], wxg[:],
                                         start=True, stop=(t == 0))
                        if t > 0:
                            for gi, (dy, dx0, Kg) in enumerate(REC_GROUPS):
                                off = o0 + dy * Ws + dx0
                                nc.tensor.matmul(
                                    ps[:, j, :], Rc[0:Kg, off:off + 128],
                                    whg[gi][:],
                                    start=False, stop=(gi == len(REC_GROUPS) - 1))
                    # ---- gates ----
                    z2 = zpool.tile([128, 3, 108], dt.float32, tag="z2")
                    nc.vector.tensor_scalar(out=z2[:], in0=ps[:, :, 0:108],
                                            scalar1=0.0, scalar2=1.0,
                                            op0=Alu.max, op1=Alu.min)
                    tslot = tcst[:, 3 * g:3 * g + 3, 0:36]
                    cslot = tcst[:, 3 * g:3 * g + 3, 36:72]
                    nc.scalar.activation(out=tslot, in_=ps[:, :, 108:144],
                                         func=Act.Tanh)
                    prod = ppool.tile([128, 3, 72], dt.float32, tag="prod")
                    nc.vector.tensor_tensor(out=prod[:], in0=z2[:, :, 0:72],
                                            in1=tcst[:, 3 * g:3 * g + 3, 0:72],
                                            op=Alu.mult)
                    nc.vector.tensor_tensor(out=cslot, in0=prod[:, :, 0:36],
                                            in1=prod[:, :, 36:72], op=Alu.add)
                    nc.scalar.activation(out=tslot, in_=cslot, func=Act.Tanh)
                    if last:
                        hf = hfpool.tile([128, 3, F], dt.float32, tag="hf")
                        nc.vector.tensor_tensor(out=hf[:], in0=z2[:, :, 72:108],
                                                in1=tslot, op=Alu.mult)
                        # hf flat order is (p, j, c); strip row = j*128 + p
                        nc.gpsimd.dma_start(
                            out=bass_mod.AP(tensor=hstage_d,
                                            offset=g * GSZ * F,
                                            ap=[[F, 128], [128 * F, 3], [1, F]]),
                            in_=hf[:])
                        continue
                    hsp = hpool.tile([128, 3, F], dt.bfloat16, tag="hsp")
                    nc.vector.tensor_tensor(out=hsp[:], in0=z2[:, :, 72:108],
                                            in1=tslot, op=Alu.mult)
                    # ---- transpose h back to channel-major into Rn ----
                    pt = tpsum.tile([F, GSZ], dt.bfloat16, tag="pt")
                    for j in range(3):
                        nc.tensor.transpose(pt[:, j * 128:(j + 1) * 128],
                                            hsp[:, j, :], ident[:])
                    m0 = ROFF + g * GSZ
                    m1 = min(m0 + GSZ, ROFF + S)
                    nc.scalar.copy(Rn[0:F, m0:m1], pt[:, 0:m1 - m0])
                    runs = garbage_runs_in(m0, m1)
                    if runs:
                        nc.vector.memset(
                            bass_mod.AP(tensor=Rn.tensor,
                                        offset=Rn.offset + runs[0],
                                        ap=[[RFREE, F], [67, len(runs)], [1, 4]]),
                            0.0)
                    for grp in (1, 2):
                        nc.sync.dma_start(
                            out=Rn[F * grp:F * (grp + 1), m0 - grp:m1 - grp],
                            in_=Rn[0:F, m0:m1])

            # ---------------- final output (chunked for overlap) ---------
            YCHUNK = 16
            for y0 in range(0, Ho, YCHUNK):
                ny = min(YCHUNK, Ho - y0)
                src = bass_mod.AP(tensor=hstage_d, offset=y0 * Ws * F,
                                  ap=[[Ws * F, ny], [1, Ho * F]])
                dst = bass_mod.AP(tensor=out_d, offset=y0 * Ho * F,
                                  ap=[[Ho * F, ny], [1, Ho * F]])
                nc.sync.dma_start(out=dst, in_=src)

    nc.compile()
    return nc


_CACHE = {}


def _get_nc():
    if "nc" not in _CACHE:
        import concourse.bass as bass_mod
        import concourse.bacc as bacc_mod
        import concourse.tile as tile_mod
        from concourse import mybir
        from concourse.masks import make_identity
        _CACHE["nc"] = build(bass_mod, bacc_mod, tile_mod, mybir, bass_mod,
                             make_identity)
    return _CACHE["nc"]


def run_on_hw(states_in, Wx, Wh, b, trace=False):
    from concourse.bass_utils import run_bass_kernel_spmd
    nc = _get_nc()
    wx = np.ascontiguousarray(Wx.reshape(75, G4), dtype=np.float32)
    wh = np.ascontiguousarray(Wh.reshape(900, G4), dtype=np.float32)
    bb = np.ascontiguousarray(b, dtype=np.float32)
    in_maps = [{
        "x": np.ascontiguousarray(states_in[i, :T], dtype=np.float32),
        "wx": wx, "wh": wh, "b": bb,
    } for i in range(B)]
    res = run_bass_kernel_spmd(nc, in_maps, core_ids=list(range(B)),
                               trace=trace)
    out = np.stack([res.results[i]["out"] for i in range(B)])
    return out, res


def kernel(states_in, Wx, Wh, b):
    out, _ = run_on_hw(states_in, Wx, Wh, b)
    return out.astype(np.float32)


# ---------------------------------------------------------------------------
# numpy simulation of the exact same dataflow (for offline logic validation)
# ---------------------------------------------------------------------------
def numpy_sim(states_in, Wx, Wh, b, bf16=True):
    import ml_dtypes

    def q(a):
        return a.astype(ml_dtypes.bfloat16).astype(np.float32) if bf16 else a

    out = np.zeros((B, Ho * Ho * F), np.float32)
    wx = Wx.reshape(75, G4).astype(np.float32)
    wh = Wh.reshape(900, G4).astype(np.float32)

    # permuted + scaled weights
    def permcols(a):
        r = np.zeros_like(a)
        for (d0, d1), (s0, s1) in COL_PERM:
            r[..., d0:d1] = a[..., s0:s1]
        return r

    wxp = permcols(wx)
    whp = permcols(wh)
    bp = permcols(b.astype(np.float32))
    brow = np.concatenate([0.2 * bp[0:108] + 0.5, bp[108:144]])
    wxg = np.vstack([wxp, brow[None, :]])
    wxg[0:75, 0:108] *= 0.2
    wxg = q(wxg)
    whgs = []
    for dy, dx0, Kg in REC_GROUPS:
        r0 = dy * 180 + dx0 * 36
        wt = whp[r0:r0 + Kg].copy()
        wt[:, 0:108] *= 0.2
        whgs.append(q(wt))

    ys, xos = np.divmod(np.arange(S), Ws)
    i0 = (4 * ys[:, None] + np.arange(KK)[None, :]) * WC + 4 * xos[:, None] * C
    gidx = i0[:, :, None] + np.arange(15)[None, None, :]  # [S, 5, 15]

    for bi in range(B):
        xflat = states_in[bi, :T].astype(np.float32).reshape(T, -1)
        R0 = [np.zeros((108, RFREE), np.float32) for _ in range(2)]
        cst = np.zeros((SPAD, F), np.float32)
        hfinal = np.zeros((SPAD, F), np.float32)

        def build_x75(t):
            X = np.zeros((76, SPAD), np.float32)
            X[75] = 1.0
            vals = q(xflat[t][gidx])            # [S, 5, 15]
            X[0:75, 0:S] = vals.reshape(S, 75).T
            return X

        for t in range(T):
            Xc = build_x75(t)
            Rc = R0[t % 2]
            Rn = R0[(t + 1) % 2]
            last = (t == T - 1)
            h_all = np.zeros((SPAD, F), np.float32)
            for blk in range(NBLK):
                o0 = blk * 128
                ps = Xc[:, o0:o0 + 128].T @ wxg
                if t > 0:
                    for gi, (dy, dx0, Kg) in enumerate(REC_GROUPS):
                        off = o0 + dy * Ws + dx0
                        ps = ps + Rc[0:Kg, off:off + 128].T @ whgs[gi]
                sig = np.clip(ps[:, 0:108], 0.0, 1.0)
                tta = np.tanh(ps[:, 108:144])
                i_g, f_g, o_g = sig[:, 0:36], sig[:, 36:72], sig[:, 72:108]
                c_old = cst[o0:o0 + 128]
                c_new = f_g * c_old + i_g * tta
                cst[o0:o0 + 128] = c_new
                h_all[o0:o0 + 128] = o_g * np.tanh(c_new)
            if last:
                hfinal = h_all
            else:
                Rn[:] = 0.0
                Rn[0:F, ROFF:ROFF + S] = q(h_all[:S]).T
                mask = np.ones(S, bool)
                mask[xos >= 63] = False
                Rn[0:F, ROFF:ROFF + S][:, ~mask] = 0.0
                Rn[F:2 * F, :-1] = Rn[0:F, 1:]
                Rn[2 * F:3 * F, :-2] = Rn[0:F, 2:]

        hf = hfinal[:S].reshape(NY, Ws, F)[:, 0:Ho, :]
        out[bi] = hf.reshape(-1)
    return out


# revision 25
# speedup vs baseline: 84.2438x; 1.0035x over previous
"""ConvLSTM2D (filters=36, kernel 5x5, strides 4, valid) + Flatten on 8 trn2 cores.

Data-parallel over batch (B=8): core i handles batch element i end-to-end.

Per-core dataflow (all shapes per core):
  x: (16, 256, 256, 3) f32.  Wx: (75, 144), Wh: (900, 144), b: (144) f32.
  Input conv (stride 4, VALID) -> (63, 63, 144) per step, computed as one
  K=76 GEMM per 128-spatial block from a DMA-gathered im2col (bias folded
  into a ones-row).  Recurrent conv (SAME, stride 1) -> 10 accumulating
  GEMMs per block from a replicated channel-major padded h buffer (halo
  trick).  Gates are evaluated spatial-major ([128 positions, 144 cols]),
  LSTM state c kept in fp32, h written back channel-major via PE transpose.

Strip layout: o = y*67 + xo, y in [0,63), xo in [0,67); xo>=63 are halo
(garbage) columns that map exactly onto the zero-padding columns of the
padded h image, so they are memset to zero after each step.
"""

import os
import sys
import numpy as np

sys.path.insert(0, "/opt/trn_rl_repo")
sys.path.insert(0, "/opt/pypackages")

# The kernel executes on the 8 axon-tunneled NeuronCores via PJRT. If the
# calling process pinned jax to cpu (common for running the jax reference)
# and jax has not been imported yet, drop the pin so the axon platform is
# discoverable when the bass runtime initializes jax.
if "jax" not in sys.modules and os.environ.get("JAX_PLATFORMS") == "cpu":
    del os.environ["JAX_PLATFORMS"]

B = 8
T = int(os.environ.get("KERNEL_T", "16"))
H = W = 256
C = 3
F = 36
KK = 5
G4 = 4 * F            # 144
Ho = (H - KK) // 4 + 1  # 63
Ws = Ho + 4             # 67 strip cols (63 valid + 4 halo)
NY = Ho                 # 63 strip rows
S = NY * Ws             # 4221
SPAD = 4224             # 33 blocks of 128
NBLK = SPAD // 128      # 33
NG = 11                 # gate groups of 3 blocks
GSZ = 384               # strip positions per gate group
WC = W * C              # 768
RFREE = 4608            # R buffer free size (needs >= 4224 + 272 + margin)
ROFF = 136              # h strip position s lands at R0 column s + 136

# recurrent tap groups: (dy, dx0, K) ; lhsT = R[0:K, o + dy*67 + dx0 : +128]
REC_GROUPS = []
for dy in range(KK):
    REC_GROUPS.append((dy, 0, 108))
    REC_GROUPS.append((dy, 3, 72))

# gate column permutation: psum cols = [i(0:36) f(36:72) o(72:108) c~(108:144)]
# original keras order in the 144 axis: [i f c o]
# dest col range -> src col range in (Wx, Wh, b)
COL_PERM = [((0, 72), (0, 72)), ((72, 108), (108, 144)), ((108, 144), (72, 108))]


def garbage_runs_in(m0, m1):
    """R0 column runs (start, 4) inside [m0, m1) holding halo strip cols."""
    runs = []
    y = max(0, (m0 - ROFF - 63 + 66) // 67)
    while True:
        a = ROFF + y * 67 + 63
        if a >= m1:
            break
        if a >= m0:
            assert a + 4 <= m1, "garbage run straddles group boundary"
            runs.append(a)
        y += 1
    return runs


def build(nc_mod, bacc_mod, tile_mod, mybir, bass_mod, make_identity):
    nc = bacc_mod.Bacc("TRN2", target_bir_lowering=False, debug=False,
                       enable_asserts=True, num_devices=B)
    dt = mybir.dt
    Alu = mybir.AluOpType
    Act = mybir.ActivationFunctionType

    x_d = nc.dram_tensor("x", [T, H, W, C], dt.float32, kind="ExternalInput")
    wx_d = nc.dram_tensor("wx", [75, G4], dt.float32, kind="ExternalInput")
    wh_d = nc.dram_tensor("wh", [900, G4], dt.float32, kind="ExternalInput")
    b_d = nc.dram_tensor("b", [G4], dt.float32, kind="ExternalInput")
    out_d = nc.dram_tensor("out", [Ho * Ho * F], dt.float32, kind="ExternalOutput")
    stage_d = nc.dram_tensor("stage", [2, SPAD, 128], dt.bfloat16)
    hstage_d = nc.dram_tensor("hstage", [SPAD, F], dt.float32)

    with tile_mod.TileContext(nc) as tc:
        import contextlib
        with contextlib.ExitStack() as ctx:
            state = ctx.enter_context(tc.tile_pool(name="state", bufs=1))
            zpool = ctx.enter_context(tc.tile_pool(name="z2", bufs=6))
            ppool = ctx.enter_context(tc.tile_pool(name="prod", bufs=6))
            hpool = ctx.enter_context(tc.tile_pool(name="hsp", bufs=6))
            hfpool = ctx.enter_context(tc.tile_pool(name="hspf", bufs=4))
            gpsum = ctx.enter_context(tc.tile_pool(name="gps", bufs=6, space="PSUM"))
            tpsum = ctx.enter_context(tc.tile_pool(name="tps", bufs=2, space="PSUM"))

            # ---------------- persistent state ----------------
            X75 = [state.tile([128, SPAD], dt.bfloat16, name=f"x75_{i}") for i in range(2)]
            R = [state.tile([108, RFREE], dt.bfloat16, name=f"R{i}") for i in range(2)]
            tcst = state.tile([128, NBLK, 72], dt.float32, tag="tc")
            ident = state.tile([128, 128], dt.bfloat16, tag="ident")
            ones = state.tile([1, SPAD], dt.bfloat16, tag="ones")
            zrow = state.tile([1, 384], dt.bfloat16, tag="zrow")

            # only the borders stay permanently zero; [ROFF-2, ROFF+S) is
            # fully rewritten every step by evac + replicas
            for Ri in R:
                nc.vector.memset(Ri[:, 0:ROFF], 0.0)
                nc.vector.memset(Ri[:, ROFF + S - 4:RFREE], 0.0)
            nc.vector.memset(tcst[:], 0.0)
            nc.vector.memset(ones[:], 1.0)
            nc.vector.memset(zrow[:], 0.0)
            make_identity(nc, ident[:])
            # zero the never-gathered tail rows (strip 4221..4223) of both
            # stage buffers: 3 rows x 128 cols = 384 bf16 elements each
            for i in range(2):
                nc.sync.dma_start(
                    out=bass_mod.AP(tensor=stage_d, offset=(i * SPAD + S) * 128,
                                    ap=[[1, (SPAD - S) * 128]]),
                    in_=zrow[0:1, 0:(SPAD - S) * 128])

            # ---------------- im2col gather helpers ----------------
            def gather(t):
                sb = t % 2
                for dy in range(KK):
                    src = bass_mod.AP(
                        tensor=x_d,
                        offset=t * H * WC + dy * WC,
                        ap=[[4 * WC, NY], [4 * C, Ws], [1, 15]])
                    dst = bass_mod.AP(
                        tensor=stage_d,
                        offset=sb * SPAD * 128 + 15 * dy,
                        ap=[[128 * Ws, NY], [128, Ws], [1, 15]])
                    nc.gpsimd.dma_start(out=dst, in_=src)

            def transpose_x(t):
                sb = t % 2
                nc.sync.dma_start(out=X75[sb][:], in_=stage_d[sb], transpose=True)
                nc.sync.dma_start(out=X75[sb][75:76, :], in_=ones[:])

            gather(0)
            transpose_x(0)

            # ---------------- weights ----------------
            # raw (unpermuted) loads; permutation done by slice ops below:
            # dest [0:72]=src[0:72] (i,f: x0.2), dest [72:108]=src[108:144]
            # (o: x0.2), dest [108:144]=src[72:108] (c~: copy)
            wxs = state.tile([76, G4], dt.float32, tag="wxs")
            wxg = state.tile([76, G4], dt.bfloat16, tag="wxg")
            nc.sync.dma_start(out=wxs[0:75, :], in_=wx_d[:])
            brow = state.tile([1, G4], dt.float32, tag="brow")
            nc.sync.dma_start(out=brow[0:1, :], in_=b_d[:].unsqueeze(0))
            browp = state.tile([1, G4], dt.float32, tag="browp")
            nc.vector.tensor_scalar(out=browp[:, 0:72], in0=brow[:, 0:72],
                                    scalar1=0.2, scalar2=0.5,
                                    op0=Alu.mult, op1=Alu.add)
            nc.vector.tensor_scalar(out=browp[:, 72:108], in0=brow[:, 108:144],
                                    scalar1=0.2, scalar2=0.5,
                                    op0=Alu.mult, op1=Alu.add)
            nc.vector.tensor_copy(browp[:, 108:144], brow[:, 72:108])
            browb = state.tile([1, G4], dt.bfloat16, tag="browb")
            nc.vector.tensor_copy(browb[:], browp[:])
            nc.vector.tensor_scalar_mul(wxg[0:75, 0:72], wxs[0:75, 0:72], 0.2)
            nc.vector.tensor_scalar_mul(wxg[0:75, 72:108], wxs[0:75, 108:144], 0.2)
            nc.vector.tensor_copy(wxg[0:75, 108:144], wxs[0:75, 72:108])
            nc.sync.dma_start(out=wxg[75:76, :], in_=browb[:])

            whg = []
            wstage = ctx.enter_context(tc.tile_pool(name="wstage", bufs=3))
            for gi, (dy, dx0, Kg) in enumerate(REC_GROUPS):
                r0 = dy * 180 + dx0 * 36
                wt = state.tile([Kg, G4], dt.bfloat16, name=f"whg{gi}")
                whs = wstage.tile([108, G4], dt.float32, tag="whs")
                nc.sync.dma_start(out=whs[0:Kg, :], in_=wh_d[r0:r0 + Kg, :])
                nc.vector.tensor_scalar_mul(wt[0:Kg, 0:72], whs[0:Kg, 0:72], 0.2)
                nc.vector.tensor_scalar_mul(wt[0:Kg, 72:108], whs[0:Kg, 108:144], 0.2)
                nc.vector.tensor_copy(wt[0:Kg, 108:144], whs[0:Kg, 72:108])
                whg.append(wt)

            # ---------------- time steps ----------------
            for t in range(T):
                Xc = X75[t % 2]
                Rc = R[t % 2]
                Rn = R[(t + 1) % 2]
                last = (t == T - 1)
                if not last:
                    gather(t + 1)
                    transpose_x(t + 1)
                for g in range(NG):
                    ps = gpsum.tile([128, 3, G4], dt.float32, tag="gps")
                    for j in range(3):
                        blk = g * 3 + j
                        o0 = blk * 128
                        nc.tensor.matmul(ps[:, j, :], Xc[0:76, o0:o0 + 128], wxg[:],
                                         start=True, stop=(t == 0))
                        if t > 0:
                            for gi, (dy, dx0, Kg) in enumerate(REC_GROUPS):
                                off = o0 + dy * Ws + dx0
                                nc.tensor.matmul(
                                    ps[:, j, :], Rc[0:Kg, off:off + 128],
                                    whg[gi][:],
                                    start=False, stop=(gi == len(REC_GROUPS) - 1))
                    # ---- gates ----
                    z2 = zpool.tile([128, 3, 108], dt.float32, tag="z2")
                    # clip = min(relu(.), 1): relu on ACT (evacuates PSUM),
                    # min on DVE in 2x single-src mode
                    nc.scalar.activation(out=z2[:], in_=ps[:, :, 0:108],
                                         func=Act.Relu)
                    nc.vector.tensor_scalar_min(z2[:], z2[:], 1.0)
                    tslot = tcst[:, 3 * g:3 * g + 3, 0:36]
                    cslot = tcst[:, 3 * g:3 * g + 3, 36:72]
                    nc.scalar.activation(out=tslot, in_=ps[:, :, 108:144],
                                         func=Act.Tanh)
                    prod = ppool.tile([128, 3, 72], dt.float32, tag="prod")
                    nc.vector.tensor_tensor(out=prod[:], in0=z2[:, :, 0:72],
                                            in1=tcst[:, 3 * g:3 * g + 3, 0:72],
                                            op=Alu.mult)
                    nc.vector.tensor_tensor(out=cslot, in0=prod[:, :, 0:36],
                                            in1=prod[:, :, 36:72], op=Alu.add)
                    nc.scalar.activation(out=tslot, in_=cslot, func=Act.Tanh)
                    if last:
                        hf = hfpool.tile([128, 3, F], dt.float32, tag="hf")
                        nc.vector.tensor_tensor(out=hf[:], in0=z2[:, :, 72:108],
                                                in1=tslot, op=Alu.mult)
                        # hf flat order is (p, j, c); strip row = j*128 + p
                        nc.gpsimd.dma_start(
                            out=bass_mod.AP(tensor=hstage_d,
                                            offset=g * GSZ * F,
                                            ap=[[F, 128], [128 * F, 3], [1, F]]),
                            in_=hf[:])
                        continue
                    hsp = hpool.tile([128, 3, F], dt.bfloat16, tag="hsp")
                    nc.vector.tensor_tensor(out=hsp[:], in0=z2[:, :, 72:108],
                                            in1=tslot, op=Alu.mult)
                    # ---- transpose h back to channel-major into Rn ----
                    pt = tpsum.tile([F, GSZ], dt.bfloat16, tag="pt")
                    for j in range(3):
                        nc.tensor.transpose(pt[:, j * 128:(j + 1) * 128],
                                            hsp[:, j, :], ident[:])
                    m0 = ROFF + g * GSZ
                    m1 = min(m0 + GSZ, ROFF + S)
                    nc.scalar.copy(Rn[0:F, m0:m1], pt[:, 0:m1 - m0])
                    runs = garbage_runs_in(m0, m1)
                    if runs:
                        nc.vector.memset(
                            bass_mod.AP(tensor=Rn.tensor,
                                        offset=Rn.offset + runs[0],
                                        ap=[[RFREE, F], [67, len(runs)], [1, 4]]),
                            0.0)
                    for grp in (1, 2):
                        nc.sync.dma_start(
                            out=Rn[F * grp:F * (grp + 1), m0 - grp:m1 - grp],
                            in_=Rn[0:F, m0:m1])

            # ---------------- final output (chunked for overlap) ---------
            YCHUNK = 16
            for y0 in range(0, Ho, YCHUNK):
                ny = min(YCHUNK, Ho - y0)
                src = bass_mod.AP(tensor=hstage_d, offset=y0 * Ws * F,
                                  ap=[[Ws * F, ny], [1, Ho * F]])
                dst = bass_mod.AP(tensor=out_d, offset=y0 * Ho * F,
                                  ap=[[Ho * F, ny], [1, Ho * F]])
                nc.sync.dma_start(out=dst, in_=src)

    nc.compile()
    return nc


_CACHE = {}


def _get_nc():
    if "nc" not in _CACHE:
        import concourse.bass as bass_mod
        import concourse.bacc as bacc_mod
        import concourse.tile as tile_mod
        from concourse import mybir
        from concourse.masks import make_identity
        _CACHE["nc"] = build(bass_mod, bacc_mod, tile_mod, mybir, bass_mod,
                             make_identity)
    return _CACHE["nc"]


def run_on_hw(states_in, Wx, Wh, b, trace=False):
    from concourse.bass_utils import run_bass_kernel_spmd
    nc = _get_nc()
    wx = np.ascontiguousarray(Wx.reshape(75, G4), dtype=np.float32)
    wh = np.ascontiguousarray(Wh.reshape(900, G4), dtype=np.float32)
    bb = np.ascontiguousarray(b, dtype=np.float32)
    in_maps = [{
        "x": np.ascontiguousarray(states_in[i, :T], dtype=np.float32),
        "wx": wx, "wh": wh, "b": bb,
    } for i in range(B)]
    res = run_bass_kernel_spmd(nc, in_maps, core_ids=list(range(B)),
                               trace=trace)
    out = np.stack([res.results[i]["out"] for i in range(B)])
    return out, res


def kernel(states_in, Wx, Wh, b):
    out, _ = run_on_hw(states_in, Wx, Wh, b)
    return out.astype(np.float32)


# ---------------------------------------------------------------------------
# numpy simulation of the exact same dataflow (for offline logic validation)
# ---------------------------------------------------------------------------
def numpy_sim(states_in, Wx, Wh, b, bf16=True):
    import ml_dtypes

    def q(a):
        return a.astype(ml_dtypes.bfloat16).astype(np.float32) if bf16 else a

    out = np.zeros((B, Ho * Ho * F), np.float32)
    wx = Wx.reshape(75, G4).astype(np.float32)
    wh = Wh.reshape(900, G4).astype(np.float32)

    # permuted + scaled weights
    def permcols(a):
        r = np.zeros_like(a)
        for (d0, d1), (s0, s1) in COL_PERM:
            r[..., d0:d1] = a[..., s0:s1]
        return r

    wxp = permcols(wx)
    whp = permcols(wh)
    bp = permcols(b.astype(np.float32))
    brow = np.concatenate([0.2 * bp[0:108] + 0.5, bp[108:144]])
    wxg = np.vstack([wxp, brow[None, :]])
    wxg[0:75, 0:108] *= 0.2
    wxg = q(wxg)
    whgs = []
    for dy, dx0, Kg in REC_GROUPS:
        r0 = dy * 180 + dx0 * 36
        wt = whp[r0:r0 + Kg].copy()
        wt[:, 0:108] *= 0.2
        whgs.append(q(wt))

    ys, xos = np.divmod(np.arange(S), Ws)
    i0 = (4 * ys[:, None] + np.arange(KK)[None, :]) * WC + 4 * xos[:, None] * C
    gidx = i0[:, :, None] + np.arange(15)[None, None, :]  # [S, 5, 15]

    for bi in range(B):
        xflat = states_in[bi, :T].astype(np.float32).reshape(T, -1)
        R0 = [np.zeros((108, RFREE), np.float32) for _ in range(2)]
        cst = np.zeros((SPAD, F), np.float32)
        hfinal = np.zeros((SPAD, F), np.float32)

        def build_x75(t):
            X = np.zeros((76, SPAD), np.float32)
            X[75] = 1.0
            vals = q(xflat[t][gidx])            # [S, 5, 15]
            X[0:75, 0:S] = vals.reshape(S, 75).T
            return X

        for t in range(T):
            Xc = build_x75(t)
            Rc = R0[t % 2]
            Rn = R0[(t + 1) % 2]
            last = (t == T - 1)
            h_all = np.zeros((SPAD, F), np.float32)
            for blk in range(NBLK):
                o0 = blk * 128
                ps = Xc[:, o0:o0 + 128].T @ wxg
                if t > 0:
                    for gi, (dy, dx0, Kg) in enumerate(REC_GROUPS):
                        off = o0 + dy * Ws + dx0
                        ps = ps + Rc[0:Kg, off:off + 128].T @ whgs[gi]
                sig = np.clip(ps[:, 0:108], 0.0, 1.0)
                tta = np.tanh(ps[:, 108:144])
                i_g, f_g, o_g = sig[:, 0:36], sig[:, 36:72], sig[:, 72:108]
                c_old = cst[o0:o0 + 128]
                c_new = f_g * c_old + i_g * tta
                cst[o0:o0 + 128] = c_new
                h_all[o0:o0 + 128] = o_g * np.tanh(c_new)
            if last:
                hfinal = h_all
            else:
                Rn[:] = 0.0
                Rn[0:F, ROFF:ROFF + S] = q(h_all[:S]).T
                mask = np.ones(S, bool)
                mask[xos >= 63] = False
                Rn[0:F, ROFF:ROFF + S][:, ~mask] = 0.0
                Rn[F:2 * F, :-1] = Rn[0:F, 1:]
                Rn[2 * F:3 * F, :-2] = Rn[0:F, 2:]

        hf = hfinal[:S].reshape(NY, Ws, F)[:, 0:Ho, :]
        out[bi] = hf.reshape(-1)
    return out
